# revision 38
# baseline (speedup 1.0000x reference)
"""Trainium2 Bass kernel for nn_MultiHeadAttention_65773129171319.

Complex-valued multi-head attention:
  attn = softmax(|Qc Kc^H| / sqrt(2 dk)) ; out = (attn @ Vr) Wo, (attn @ Vp) Wo

Sharding: 8 cores = 2 (batch) x 4 (head-groups of 2 heads).  Each core
computes its batch's full sequence for its 2 heads; the out-projection
partial sums (over head groups) are reduced on the host.

Device pipeline (per core; strips of 512 q-positions, blocks = (strip, head)):
  - scores come out TRANSPOSED [sk, sq] from stacked-channel matmuls
    (kcr=[Kr;-Kp], kcp=[Kp;Kr] vs qc=[Qr;Qp], contraction 128), in PAIRS of
    sk-tiles sharing a 2-bank PSUM tile to amortise PSUM access latency.
  - u = s_r^2 (ACT Square or DVE SQ1, balanced) then u += s_p^2 (DVE SQADD).
  - sqrt / exp batched per strip on ACT; Square/Sqrt/Copy share one table
    set so only the exp<->sqrt switch reloads tables.
  - consume(strip-1) — 16 rowsum matmuls FIRST (so 1/Z is ready early),
    then 16 merged-AV matmuls (stationary [vr|vp], M=128) — is interleaved
    into the next strip's score matmuls so the PE never idles during the
    ACT/DVE chain; normalisation + out-projection follow.
  - head-1 AV stationary is column-swapped ([vp|vr]) so every DVE op stays
    partition-aligned; the phase out-projection uses a row-swapped Wo.
  - for the repeat-loop build, the K/Q(0) projections are software-pipelined
    ACROSS iterations: emitted once before For_i, then re-emitted at the
    body tail where they overlap the attention drain.
"""

import sys

import numpy as np

try:
    import concourse.bass as bass
except ImportError:  # pragma: no cover
    sys.path.insert(0, "/opt/trn_rl_repo")
    import concourse.bass as bass

import ml_dtypes
import concourse.mybir as mybir
import concourse.tile as tile
from concourse import bacc
from concourse.bass_utils import run_bass_kernel_spmd

B, S, D, H = 2, 2048, 512, 8
DK = D // H  # 64
SCALE = float((2 * DK) ** 0.5)
P = 128
N_CORES = 8
HG = 4            # head groups (2 heads each)
DT = D // P       # 4 d-tiles for projection contraction
SKT = S // P      # 16 sk tiles
NSTRIP = 4        # sq strips of 512
STRIP = S // NSTRIP  # 512
SQT_ACT = 8       # of the 16 sk-tiles per block, how many square on ACT

F32 = mybir.dt.float32
BF16 = mybir.dt.bfloat16
BFNP = ml_dtypes.bfloat16

AF = mybir.ActivationFunctionType


def register_custom_ops():
    """Register fused DVE ops (runtime extension of dve_ops.OPS)."""
    import concourse.dve_ops as dve_ops
    from concourse.dve_ops import DveOp
    from concourse.dve_spec import Spec, Src0, Src1, sq, lower, _has_src1
    from concourse.dve_uop import DveOpSpec

    existing = {op.name: op for op in dve_ops.OPS}

    def mk(name, spec):
        if name in existing:
            return existing[name]
        row = max(dve_ops._SUB_OPCODE_FOR_NAME.values()) + 1
        assert row < 0x20, "no free DVE opcode rows"
        dve_ops._SUB_OPCODE_FOR_NAME[name] = row
        shas = {}
        for ver in ("v3", "v4"):
            s = DveOpSpec(name=name, opcode=row, uops=lower(spec, ver=ver),
                          rd1_en=_has_src1(spec))
            shas[ver] = s.sha(ver)
        op = DveOp(name, spec, subdim=False, uops_sha=shas)
        dve_ops.OPS.append(op)
        return op

    sq1 = mk("SQ1_ANT", Spec(
        body=sq(Src0),
        reference=lambda in0, in1, s0, s1, imm2: in0.astype(np.float32) ** 2))
    sqadd = mk("SQADD_ANT", Spec(
        body=sq(Src0) + Src1,
        reference=lambda in0, in1, s0, s1, imm2:
            in0.astype(np.float32) ** 2 + in1.astype(np.float32)))
    return sq1, sqadd


SQ1, SQADD = register_custom_ops()


def build(n_iter: int = 1, variant: frozenset = frozenset(),
          unroll_wrap: bool = False):
    """Build (and bacc-compile) the per-core SPMD program."""
    nc = bacc.Bacc("TRN2", target_bir_lowering=False, debug=False,
                   num_devices=N_CORES)

    dr = {}
    for name in ("xqr", "xqp", "xkr", "xkp", "xvr", "xvp"):
        dr[name] = nc.dram_tensor(name, [D, S], BF16, kind="ExternalInput")
    for name in ("wq", "wk", "wv"):
        dr[name] = nc.dram_tensor(name, [D, 2 * DK], BF16, kind="ExternalInput")
    dr["wo"] = nc.dram_tensor("wo", [2 * DK, D], BF16, kind="ExternalInput")
    dr["wop"] = nc.dram_tensor("wop", [2 * DK, D], BF16, kind="ExternalInput")
    dr["o_r"] = nc.dram_tensor("o_r", [S, D], BF16, kind="ExternalOutput")
    dr["o_p"] = nc.dram_tensor("o_p", [S, D], BF16, kind="ExternalOutput")

    with tile.TileContext(nc) as tc:
        _emit(tc, dr, n_iter, variant, unroll_wrap)
    nc.compile()
    return nc


def _emit(tc, dr, n_iter, variant=frozenset(), unroll_wrap=False):
    from contextlib import ExitStack

    ctx = ExitStack()
    with ctx:
        pools = dict(
            singles=ctx.enter_context(tc.tile_pool(name="singles", bufs=1)),
            xpool=ctx.enter_context(tc.tile_pool(name="xp", bufs=4)),
            upool=ctx.enter_context(tc.tile_pool(name="up", bufs=4)),
            tpool=ctx.enter_context(tc.tile_pool(name="tp", bufs=2)),
            opool=ctx.enter_context(tc.tile_pool(name="op", bufs=4)),
            psA=ctx.enter_context(tc.tile_pool(name="psA", bufs=4, space="PSUM")),
            psAV=ctx.enter_context(tc.tile_pool(name="psAV", bufs=2, space="PSUM")),
            psRS=ctx.enter_context(tc.tile_pool(name="psRS", bufs=2, space="PSUM")),
        )
        kb = _KernelBody(tc, dr, variant, **pools)
        kb.weights_and_persistent()
        kb.kq_lead()
        if n_iter > 1 and unroll_wrap:
            kb.wrap_prologue()
            for _ in range(n_iter):
                kb.body(trail_proj=True, wrap=True)
        elif n_iter > 1:
            kb.wrap_prologue()
            # unroll inside For_i to amortise its per-iteration all-engine
            # barrier (which would otherwise cut the cross-iteration
            # software pipeline).
            unroll = max(u for u in (8, 6, 5, 4, 3, 2, 1) if n_iter % u == 0)
            with tc.For_i(0, n_iter // unroll, 1):
                for _ in range(unroll):
                    kb.body(trail_proj=True, wrap=True)
        else:
            kb.body(trail_proj=False, wrap=False)


class _KernelBody:
    def __init__(self, tc, dr, variant, singles, xpool, upool, tpool, opool,
                 psA, psAV, psRS):
        self.tc = tc
        self.nc = tc.nc
        self.dr = dr
        self.variant = variant
        self.singles = singles
        self.xpool = xpool
        self.upool = upool
        self.tpool = tpool
        self.opool = opool
        self.psA = psA
        self.psAV = psAV
        self.psRS = psRS

    # ---- one-time setup --------------------------------------------------
    def weights_and_persistent(self):
        nc, dr, singles = self.nc, self.dr, self.singles
        self.wsb = {}
        for name in ("wq", "wk", "wv"):
            t = singles.tile([P, DT, 2 * DK], BF16, tag=f"w_{name}",
                             name=f"w_{name}")
            nc.sync.dma_start(out=t[:],
                              in_=dr[name].rearrange("(dt p) m -> p dt m", p=P))
            self.wsb[name] = t
        self.wkn = singles.tile([P, DT, 2 * DK], BF16, tag="w_wkn", name="w_wkn")
        nc.scalar.mul(out=self.wkn[:], in_=self.wsb["wk"][:], mul=-1.0)
        self.wo = singles.tile([P, D], BF16, tag="w_wo", name="w_wo")
        nc.sync.dma_start(out=self.wo[:], in_=dr["wo"][:])
        self.wop = singles.tile([P, D], BF16, tag="w_wop", name="w_wop")
        nc.sync.dma_start(out=self.wop[:], in_=dr["wop"][:])
        self.ones = singles.tile([P, 1], BF16, tag="ones", name="ones")
        nc.vector.memset(self.ones[:], 1.0)

        # persistent: kc2[h][:,0,:]=kcr=[Kr;-Kp], [:,1,:]=kcp=[Kp;Kr]
        self.kc2 = [singles.tile([P, 2, S], BF16, tag=f"kc{h}", name=f"kc{h}")
                    for h in range(2)]
        self.qc2 = singles.tile([P, 2, S], BF16, tag="qc2", name="qc2")
        self.vs = [singles.tile([P, SKT, 2 * DK], BF16, tag=f"vs{h}",
                                name=f"vs{h}") for h in range(2)]
        self.xr2hT = singles.tile([P, S], BF16, tag="xr2hT", name="xr2hT")
        self.xp2hT = singles.tile([P, S], BF16, tag="xp2hT", name="xp2hT")

        if "noproj" in self.variant:
            for t in self.kc2 + self.vs + [self.qc2]:
                nc.vector.memset(t[:], 0.01)
        if "noav" in self.variant:
            nc.vector.memset(self.xr2hT[:], 0.01)
            nc.vector.memset(self.xp2hT[:], 0.01)

    def _xdma(self, out, in_):
        if "nodma" not in self.variant:
            self.nc.sync.dma_start(out=out, in_=in_)

    # ---- projections -----------------------------------------------------
    def kq_lead(self):
        """K projection (all strips) + Q projection (strip 0)."""
        if "noproj" in self.variant:
            return
        nc = self.nc
        for s in range(NSTRIP):
            ssl = slice(s * STRIP, (s + 1) * STRIP)
            xtr = self.xpool.tile([P, DT, STRIP], BF16, tag="xs", name="xs")
            self._xdma(xtr[:], self.dr["xkr"].rearrange(
                "(dt p) s -> p dt s", p=P)[:, :, ssl])
            xtp = self.xpool.tile([P, DT, STRIP], BF16, tag="xs", name="xs")
            self._xdma(xtp[:], self.dr["xkp"].rearrange(
                "(dt p) s -> p dt s", p=P)[:, :, ssl])
            for h in range(2):
                hsl = slice(h * DK, (h + 1) * DK)
                pkr = self.psA.tile([P, STRIP], F32, tag="psA", name="psA")
                pkp = self.psA.tile([P, STRIP], F32, tag="psA", name="psA")
                for dt in range(DT):
                    st = (dt == 0)
                    sp = (dt == DT - 1)
                    # kcr = [Kr ; -Kp]
                    nc.tensor.matmul(pkr[0:DK, :], self.wsb["wk"][:, dt, hsl],
                                     xtr[:, dt, :], start=st, stop=sp)
                    nc.tensor.matmul(pkr[DK:P, :], self.wkn[:, dt, hsl],
                                     xtp[:, dt, :], start=st, stop=sp)
                    # kcp = [Kp ; Kr]
                    nc.tensor.matmul(pkp[0:DK, :], self.wsb["wk"][:, dt, hsl],
                                     xtp[:, dt, :], start=st, stop=sp)
                    nc.tensor.matmul(pkp[DK:P, :], self.wsb["wk"][:, dt, hsl],
                                     xtr[:, dt, :], start=st, stop=sp)
                nc.vector.tensor_copy(self.kc2[h][:, 0, ssl], pkr[:])
                nc.vector.tensor_copy(self.kc2[h][:, 1, ssl], pkp[:])
        self.qproj(0)

    def qproj(self, s):
        if "noproj" in self.variant:
            return
        nc = self.nc
        ssl = slice(s * STRIP, (s + 1) * STRIP)
        xtr = self.xpool.tile([P, DT, STRIP], BF16, tag="xs", name="xs")
        self._xdma(xtr[:], self.dr["xqr"].rearrange(
            "(dt p) s -> p dt s", p=P)[:, :, ssl])
        xtp = self.xpool.tile([P, DT, STRIP], BF16, tag="xs", name="xs")
        self._xdma(xtp[:], self.dr["xqp"].rearrange(
            "(dt p) s -> p dt s", p=P)[:, :, ssl])
        for h in range(2):
            hsl = slice(h * DK, (h + 1) * DK)
            pq = self.psA.tile([P, STRIP], F32, tag="psA", name="psA")
            for dt in range(DT):
                st = (dt == 0)
                sp = (dt == DT - 1)
                nc.tensor.matmul(pq[0:DK, :], self.wsb["wq"][:, dt, hsl],
                                 xtr[:, dt, :], start=st, stop=sp)
                nc.tensor.matmul(pq[DK:P, :], self.wsb["wq"][:, dt, hsl],
                                 xtp[:, dt, :], start=st, stop=sp)
            nc.vector.tensor_copy(self.qc2[:, h, ssl], pq[:])

    def vproj(self):
        if "noproj" in self.variant:
            return
        nc = self.nc
        for s in range(NSTRIP):
            ts = slice(s * (STRIP // P), (s + 1) * (STRIP // P))
            for kind, srcn in ((0, "xvr"), (1, "xvp")):
                xt = self.xpool.tile([P, DT, STRIP], BF16, tag="xs", name="xs")
                self._xdma(xt[:], self.dr[srcn].rearrange(
                    "(dt p) s -> p dt s", p=P)[:, :, s * STRIP:(s + 1) * STRIP])
                vv = self.psA.tile([P, STRIP // P, P], F32, tag="psA",
                                   name="psA")
                for tt in range(STRIP // P):
                    for dt in range(DT):
                        nc.tensor.matmul(vv[:, tt, :],
                                         xt[:, dt, tt * P:(tt + 1) * P],
                                         self.wsb["wv"][:, dt, :],
                                         start=(dt == 0), stop=(dt == DT - 1))
                # vs[0] = [vr_h0 | vp_h0] ; vs[1] = [vp_h1 | vr_h1]
                nc.vector.tensor_copy(
                    self.vs[0][:, ts, kind * DK:(kind + 1) * DK],
                    vv[:, :, 0:DK])
                nc.vector.tensor_copy(
                    self.vs[1][:, ts, (1 - kind) * DK:(2 - kind) * DK],
                    vv[:, :, DK:P])

    # ---- attention pipeline ---------------------------------------------
    def _mk_exp(self, u, c):
        def emit():
            csl = slice(c * (STRIP // 4), (c + 1) * (STRIP // 4))
            self.nc.scalar.activation(u[:, :, :, csl], u[:, :, :, csl],
                                      AF.Exp, scale=1.0 / SCALE)
        return emit

    def consume_mms(self, sp, h, j):
        """Interleaved consume slot j (0..SKT-1) for block (sp, h):
        slots 0-7 carry the 16 rowsum matmuls, slots 8-15 the 16 AV."""
        nc, variant = self.nc, self.variant
        st = self.state[sp]
        if j == 0:
            if "nors" not in variant:
                st.setdefault("rs", {})[h] = self.psRS.tile(
                    [1, STRIP], F32, tag="rs", name="rs")
        if j == 0:
            if "noav" not in variant:
                st["av"][h] = self.psAV.tile([P, STRIP], F32, tag="av",
                                             name="av")
        pu = st["u"]
        if j < 8:
            if "nors" in variant:
                return
            for tt in range(2):
                t = j * 2 + tt
                nc.tensor.matmul(st["rs"][h][0:1, :], self.ones[:],
                                 pu[:, h, t, :],
                                 start=(t == 0), stop=(t == SKT - 1),
                                 skip_group_check=True)
        else:
            if j == 8:
                self.recip_bcast(sp, h)
            if "noav" in variant:
                return
            for tt in range(2):
                t = (j - 8) * 2 + tt
                nc.tensor.matmul(st["av"][h][:], self.vs[h][:, t, :],
                                 pu[:, h, t, :],
                                 start=(t == 0), stop=(t == SKT - 1),
                                 skip_group_check=True)
            if j == SKT - 1:
                self.norm(sp, h)

    def recip_bcast(self, sp, h):
        nc = self.nc
        st = self.state[sp]
        rb = self.tpool.tile([P, STRIP], F32, tag="rb", name="rb")
        st["rb"][h] = rb
        if "nors" in self.variant:
            nc.vector.memset(rb[:], 1.0)
        else:
            rrec = self.tpool.tile([1, STRIP], F32, tag="rrec", name="rrec")
            nc.vector.reciprocal_approx_fast(rrec[:], st["rs"][h][0:1, :])
            nc.gpsimd.partition_broadcast(rb[:], rrec[:])

    def norm(self, sp, h):
        nc, variant = self.nc, self.variant
        pssl = slice(sp * STRIP, (sp + 1) * STRIP)
        st = self.state[sp]
        rb = st["rb"][h]
        if "noav" not in variant:
            av = st["av"][h]
            if h == 0:   # av = [xr_h0 ; xp_h0]
                nc.vector.tensor_mul(self.xr2hT[0:DK, pssl], av[0:DK, :],
                                     rb[0:DK, :])
                nc.vector.tensor_mul(self.xp2hT[DK:P, pssl], av[DK:P, :],
                                     rb[DK:P, :])
            else:        # av = [xp_h1 ; xr_h1]
                nc.vector.tensor_mul(self.xp2hT[0:DK, pssl], av[0:DK, :],
                                     rb[0:DK, :])
                nc.vector.tensor_mul(self.xr2hT[DK:P, pssl], av[DK:P, :],
                                     rb[DK:P, :])

    def tail(self, sp):
        """Out-projection for strip sp (both heads already normalised)."""
        nc, variant = self.nc, self.variant
        pssl = slice(sp * STRIP, (sp + 1) * STRIP)
        st = self.state[sp]
        if "noout" in variant:
            return
        for kind, xT, w, out in ((0, self.xr2hT, self.wo, self.dr["o_r"]),
                                 (1, self.xp2hT, self.wop, self.dr["o_p"])):
            for qq in range(STRIP // P):
                q = sp * (STRIP // P) + qq
                qsl = slice(q * P, (q + 1) * P)
                ps_o = self.psAV.tile([P, D], F32, tag="av", name="av")
                nc.tensor.matmul(ps_o[:], xT[:, qsl], w[:], start=True,
                                 stop=True)
                osb = self.opool.tile([P, D], BF16, tag="osb", name="osb")
                if kind == 0:
                    nc.vector.tensor_copy(osb[:], ps_o[:])
                else:
                    nc.scalar.copy(osb[:], ps_o[:])
                nc.sync.dma_start(out=out[qsl, :], in_=osb[:])

    def wrap_prologue(self):
        """Pre-create all per-strip u tiles (static buffer binding across
        For_i iterations) and initialise the two consumed by the first
        iteration's wrapped pipeline stages."""
        self.state = {}
        for s in range(NSTRIP):
            u = self.upool.tile([P, 2, SKT, STRIP], BF16, tag="u", name="u")
            self.state[s] = {"u": u, "av": {}, "rb": {}}
        for s in (NSTRIP - 2, NSTRIP - 1):
            self.nc.vector.memset(self.state[s]["u"][:], 0.25)

    def body(self, trail_proj, wrap):
        nc, variant = self.nc, self.variant
        inv_scale = 1.0 / SCALE
        if not wrap:
            self.state = {}
        self.pending_act = []
        if wrap and "nosqrtexp" not in variant:
            # previous iteration's strip-3 exp chunks run spaced through
            # this iteration's strip 0 (its sqrt ran during the K/Q trail)
            self.pending_act.extend(
                self._mk_exp(self.state[NSTRIP - 1]["u"], c) for c in range(4))

        for s in range(NSTRIP):
            ssl = slice(s * STRIP, (s + 1) * STRIP)
            if wrap:
                sc = (s - 2) % NSTRIP
                u = self.state[s]["u"]
            else:
                sc = s - 2   # strip consumed while strip s computes (2-stage)
                u = self.upool.tile([P, 2, SKT, STRIP], BF16, tag="u",
                                    name="u")
                self.state[s] = {"u": u, "av": {}, "rb": {}}
            if "nosq" in variant:
                nc.vector.memset(u[:], 0.25)
            for h in range(2):
                for t in range(SKT):
                    if h == 0 and t % 4 == 2 and self.pending_act:
                        self.pending_act.pop(0)()
                    if "nosq" not in variant:
                        tsl = slice(t * P, (t + 1) * P)
                        ut = u[:, h, t, :]
                        ps_r = self.psA.tile([P, STRIP], F32, tag="psA",
                                             name="psA")
                        nc.tensor.matmul(ps_r[:], self.kc2[h][:, 0, tsl],
                                         self.qc2[:, h, ssl], start=True,
                                         stop=True)
                        ps_p = self.psA.tile([P, STRIP], F32, tag="psA",
                                             name="psA")
                        nc.tensor.matmul(ps_p[:], self.kc2[h][:, 1, tsl],
                                         self.qc2[:, h, ssl], start=True,
                                         stop=True)
                        if (t * SQT_ACT) % 16 < SQT_ACT:
                            nc.scalar.square(ut, ps_r[:])
                        else:
                            nc.vector._custom_dve(SQ1, out=ut, in0=ps_r[:])
                        nc.vector._custom_dve(SQADD, out=ut, in0=ps_p[:],
                                              in1=ut)
                    if sc is not None and sc >= 0:
                        self.consume_mms(sc, h, t)
            if "nosqrtexp" not in variant:
                # sqrt chunks emitted now (readiness staggers them against
                # the next strip's squares); exp chunks are column-sliced
                # (each depends on ALL sqrt chunks -> no table thrash) and
                # their emission is deferred into the next strip's t-loop
                # so they cannot convoy on the ACT engine.
                for c in range(4):
                    nc.scalar.activation(u[:, :, 4 * c:4 * c + 4, :],
                                         u[:, :, 4 * c:4 * c + 4, :], AF.Sqrt)
                if not (wrap and s == NSTRIP - 1):
                    self.pending_act.extend(
                        self._mk_exp(u, c) for c in range(4))
            if sc is not None and sc >= 0:
                self.tail(sc)
            if s == 0:
                self.vproj()
            if s + 1 < NSTRIP:
                self.qproj(s + 1)

        if wrap:
            # strips 2,3 are consumed by the next iteration's strips 0,1;
            # strip 3's sqrt runs during the K/Q trail, its exp inside the
            # next iteration's strip 0.
            assert not self.pending_act
            if trail_proj:
                self.kq_lead()
            return
        for emit in self.pending_act:
            emit()
        self.pending_act = []
        # next iteration's K/Q(0) projections overlap the drain below
        if trail_proj:
            self.kq_lead()
        for sc in (NSTRIP - 2, NSTRIP - 1):
            for h in range(2):
                for j in range(SKT):
                    self.consume_mms(sc, h, j)
            self.tail(sc)


# ---------------------------------------------------------------------------
_CACHE = {}


def _get_nc(n_iter=1, variant=frozenset()):
    key = (n_iter, variant)
    if key not in _CACHE:
        _CACHE[key] = build(n_iter, variant)
    return _CACHE[key]


def make_in_maps(q_real, k_real, v_real, q_phase, k_phase, v_phase,
                 w_q, w_k, w_v, w_o):
    """Host-side shard + layout prep: per-core input dicts."""
    xt = {}
    for b in range(B):
        xt[("xqr", b)] = np.ascontiguousarray(q_real[b].T).astype(BFNP)
        xt[("xqp", b)] = np.ascontiguousarray(q_phase[b].T).astype(BFNP)
        xt[("xkr", b)] = np.ascontiguousarray(k_real[b].T).astype(BFNP)
        xt[("xkp", b)] = np.ascontiguousarray(k_phase[b].T).astype(BFNP)
        xt[("xvr", b)] = np.ascontiguousarray(v_real[b].T).astype(BFNP)
        xt[("xvp", b)] = np.ascontiguousarray(v_phase[b].T).astype(BFNP)
    wq16, wk16, wv16, wo16 = (w.astype(BFNP) for w in (w_q, w_k, w_v, w_o))
    in_maps = []
    for core in range(N_CORES):
        b, hg = divmod(core, HG)
        csl = slice(hg * 2 * DK, (hg + 1) * 2 * DK)
        wo_c = np.ascontiguousarray(wo16[csl, :])
        wop_c = np.ascontiguousarray(
            np.concatenate([wo_c[DK:2 * DK], wo_c[0:DK]], axis=0))
        in_maps.append({
            "xqr": xt[("xqr", b)], "xqp": xt[("xqp", b)],
            "xkr": xt[("xkr", b)], "xkp": xt[("xkp", b)],
            "xvr": xt[("xvr", b)], "xvp": xt[("xvp", b)],
            "wq": np.ascontiguousarray(wq16[:, csl]),
            "wk": np.ascontiguousarray(wk16[:, csl]),
            "wv": np.ascontiguousarray(wv16[:, csl]),
            "wo": wo_c,
            "wop": wop_c,
        })
    return in_maps


def gather_outputs(results):
    out_r = np.zeros((B, S, D), np.float32)
    out_p = np.zeros((B, S, D), np.float32)
    for core in range(N_CORES):
        b = core // HG
        out_r[b] += np.asarray(results[core]["o_r"], np.float32)
        out_p[b] += np.asarray(results[core]["o_p"], np.float32)
    return out_r, out_p


def _numpy_fallback(q_real, k_real, v_real, q_phase, k_phase, v_phase,
                    w_q, w_k, w_v, w_o, mask):
    def heads(x, w):
        y = x @ w
        return y.reshape(B, -1, H, DK).transpose(0, 2, 1, 3)
    qr, kr, vr = heads(q_real, w_q), heads(k_real, w_k), heads(v_real, w_v)
    qp, kp, vp = heads(q_phase, w_q), heads(k_phase, w_k), heads(v_phase, w_v)
    ar = np.einsum('bhqd,bhkd->bhqk', qr, kr) - np.einsum('bhqd,bhkd->bhqk', qp, kp)
    ap = np.einsum('bhqd,bhkd->bhqk', qr, kp) + np.einsum('bhqd,bhkd->bhqk', qp, kr)
    a = np.sqrt(ar * ar + ap * ap) / SCALE
    a = np.where(mask[:, None, :, :] == 0, np.float32(-1e9), a)
    a = a - a.max(axis=-1, keepdims=True)
    e = np.exp(a)
    a = e / e.sum(axis=-1, keepdims=True)
    xr = np.einsum('bhqk,bhkd->bhqd', a, vr).transpose(0, 2, 1, 3).reshape(B, -1, D)
    xp = np.einsum('bhqk,bhkd->bhqd', a, vp).transpose(0, 2, 1, 3).reshape(B, -1, D)
    return (xr @ w_o).astype(np.float32), (xp @ w_o).astype(np.float32)


def kernel(q_real, k_real, v_real, q_phase, k_phase, v_phase,
           w_q, w_k, w_v, w_o, mask):
    args = [np.asarray(a, np.float32) for a in
            (q_real, k_real, v_real, q_phase, k_phase, v_phase,
             w_q, w_k, w_v, w_o)]
    mask = np.asarray(mask)
    if not np.all(mask != 0):
        return _numpy_fallback(*args, mask)
    nc = _get_nc(1)
    in_maps = make_in_maps(*args)
    res = run_bass_kernel_spmd(nc, in_maps, core_ids=list(range(N_CORES)))
    return gather_outputs(res.results)


# revision 45
# speedup vs baseline: 1.0634x; 1.0634x over previous
"""Trainium2 Bass kernel for nn_MultiHeadAttention_65773129171319.

Complex-valued multi-head attention:
  attn = softmax(|Qc Kc^H| / sqrt(2 dk)) ; out = (attn @ Vr) Wo, (attn @ Vp) Wo

Sharding: 8 cores = 2 (batch) x 4 (head-groups of 2 heads).  Each core
computes its batch's full sequence for its 2 heads; the out-projection
partial sums (over head groups) are reduced on the host.

Device pipeline (per core; strips of 512 q-positions, blocks = (strip, head)):
  - scores come out TRANSPOSED [sk, sq] from stacked-channel matmuls
    (kcr=[Kr;-Kp], kcp=[Kp;Kr] vs qc=[Qr;Qp], contraction 128), in PAIRS of
    sk-tiles sharing a 2-bank PSUM tile to amortise PSUM access latency.
  - u = s_r^2 (ACT Square or DVE SQ1, balanced) then u += s_p^2 (DVE SQADD).
  - sqrt / exp batched per strip on ACT; Square/Sqrt/Copy share one table
    set so only the exp<->sqrt switch reloads tables.
  - consume(strip-1) — 16 rowsum matmuls FIRST (so 1/Z is ready early),
    then 16 merged-AV matmuls (stationary [vr|vp], M=128) — is interleaved
    into the next strip's score matmuls so the PE never idles during the
    ACT/DVE chain; normalisation + out-projection follow.
  - head-1 AV stationary is column-swapped ([vp|vr]) so every DVE op stays
    partition-aligned; the phase out-projection uses a row-swapped Wo.
  - for the repeat-loop build, the K/Q(0) projections are software-pipelined
    ACROSS iterations: emitted once before For_i, then re-emitted at the
    body tail where they overlap the attention drain.
"""

import sys

import numpy as np

try:
    import concourse.bass as bass
except ImportError:  # pragma: no cover
    sys.path.insert(0, "/opt/trn_rl_repo")
    import concourse.bass as bass

import ml_dtypes
import concourse.mybir as mybir
import concourse.tile as tile
from concourse import bacc
from concourse.bass_utils import run_bass_kernel_spmd

B, S, D, H = 2, 2048, 512, 8
DK = D // H  # 64
SCALE = float((2 * DK) ** 0.5)
P = 128
N_CORES = 8
HG = 4            # head groups (2 heads each)
DT = D // P       # 4 d-tiles for projection contraction
SKT = S // P      # 16 sk tiles
NSTRIP = 4        # sq strips of 512
STRIP = S // NSTRIP  # 512
SQT_ACT = 6       # of the 16 sk-tiles per block, how many square on ACT

F32 = mybir.dt.float32
BF16 = mybir.dt.bfloat16
BFNP = ml_dtypes.bfloat16

AF = mybir.ActivationFunctionType


def register_custom_ops():
    """Register fused DVE ops (runtime extension of dve_ops.OPS)."""
    import concourse.dve_ops as dve_ops
    from concourse.dve_ops import DveOp
    from concourse.dve_spec import Spec, Src0, Src1, sq, lower, _has_src1
    from concourse.dve_uop import DveOpSpec

    existing = {op.name: op for op in dve_ops.OPS}

    def mk(name, spec):
        if name in existing:
            return existing[name]
        row = max(dve_ops._SUB_OPCODE_FOR_NAME.values()) + 1
        assert row < 0x20, "no free DVE opcode rows"
        dve_ops._SUB_OPCODE_FOR_NAME[name] = row
        shas = {}
        for ver in ("v3", "v4"):
            s = DveOpSpec(name=name, opcode=row, uops=lower(spec, ver=ver),
                          rd1_en=_has_src1(spec))
            shas[ver] = s.sha(ver)
        op = DveOp(name, spec, subdim=False, uops_sha=shas)
        dve_ops.OPS.append(op)
        return op

    sq1 = mk("SQ1_ANT", Spec(
        body=sq(Src0),
        reference=lambda in0, in1, s0, s1, imm2: in0.astype(np.float32) ** 2))
    sqadd = mk("SQADD_ANT", Spec(
        body=sq(Src0) + Src1,
        reference=lambda in0, in1, s0, s1, imm2:
            in0.astype(np.float32) ** 2 + in1.astype(np.float32)))
    return sq1, sqadd


SQ1, SQADD = register_custom_ops()


def build(n_iter: int = 1, variant: frozenset = frozenset(),
          unroll_wrap: bool = False):
    """Build (and bacc-compile) the per-core SPMD program."""
    nc = bacc.Bacc("TRN2", target_bir_lowering=False, debug=False,
                   num_devices=N_CORES)

    dr = {}
    for name in ("xqr", "xqp", "xkr", "xkp", "xvr", "xvp"):
        dr[name] = nc.dram_tensor(name, [D, S], BF16, kind="ExternalInput")
    for name in ("wq", "wk", "wv"):
        dr[name] = nc.dram_tensor(name, [D, 2 * DK], BF16, kind="ExternalInput")
    dr["wo"] = nc.dram_tensor("wo", [2 * DK, D], BF16, kind="ExternalInput")
    dr["wop"] = nc.dram_tensor("wop", [2 * DK, D], BF16, kind="ExternalInput")
    dr["o_r"] = nc.dram_tensor("o_r", [S, D], BF16, kind="ExternalOutput")
    dr["o_p"] = nc.dram_tensor("o_p", [S, D], BF16, kind="ExternalOutput")

    with tile.TileContext(nc) as tc:
        _emit(tc, dr, n_iter, variant, unroll_wrap)
    nc.compile()
    return nc


def _emit(tc, dr, n_iter, variant=frozenset(), unroll_wrap=False):
    from contextlib import ExitStack

    ctx = ExitStack()
    with ctx:
        pools = dict(
            singles=ctx.enter_context(tc.tile_pool(name="singles", bufs=1)),
            xpool=ctx.enter_context(tc.tile_pool(name="xp", bufs=4)),
            upool=ctx.enter_context(tc.tile_pool(name="up", bufs=4)),
            tpool=ctx.enter_context(tc.tile_pool(name="tp", bufs=2)),
            opool=ctx.enter_context(tc.tile_pool(name="op", bufs=4)),
            psA=ctx.enter_context(tc.tile_pool(name="psA", bufs=4, space="PSUM")),
            psAV=ctx.enter_context(tc.tile_pool(name="psAV", bufs=2, space="PSUM")),
            psRS=ctx.enter_context(tc.tile_pool(name="psRS", bufs=2, space="PSUM")),
        )
        kb = _KernelBody(tc, dr, variant, **pools)
        kb.weights_and_persistent()
        kb.kq_lead()
        if n_iter > 1 and unroll_wrap:
            kb.wrap_prologue()
            for _ in range(n_iter):
                kb.body(trail_proj=True, wrap=True)
        elif n_iter > 1:
            kb.wrap_prologue()
            # unroll inside For_i to amortise its per-iteration all-engine
            # barrier (which would otherwise cut the cross-iteration
            # software pipeline).
            unroll = max(u for u in (4, 3, 2, 1) if n_iter % u == 0)
            with tc.For_i(0, n_iter // unroll, 1):
                for _ in range(unroll):
                    kb.body(trail_proj=True, wrap=True)
        else:
            kb.body(trail_proj=False, wrap=False)


class _KernelBody:
    def __init__(self, tc, dr, variant, singles, xpool, upool, tpool, opool,
                 psA, psAV, psRS):
        self.tc = tc
        self.nc = tc.nc
        self.dr = dr
        self.variant = variant
        self.singles = singles
        self.xpool = xpool
        self.upool = upool
        self.tpool = tpool
        self.opool = opool
        self.psA = psA
        self.psAV = psAV
        self.psRS = psRS

    # ---- one-time setup --------------------------------------------------
    def weights_and_persistent(self):
        nc, dr, singles = self.nc, self.dr, self.singles
        self.wsb = {}
        for name in ("wq", "wk", "wv"):
            t = singles.tile([P, DT, 2 * DK], BF16, tag=f"w_{name}",
                             name=f"w_{name}")
            nc.sync.dma_start(out=t[:],
                              in_=dr[name].rearrange("(dt p) m -> p dt m", p=P))
            self.wsb[name] = t
        self.wkn = singles.tile([P, DT, 2 * DK], BF16, tag="w_wkn", name="w_wkn")
        nc.scalar.mul(out=self.wkn[:], in_=self.wsb["wk"][:], mul=-1.0)
        self.wo = singles.tile([P, D], BF16, tag="w_wo", name="w_wo")
        nc.sync.dma_start(out=self.wo[:], in_=dr["wo"][:])
        self.wop = singles.tile([P, D], BF16, tag="w_wop", name="w_wop")
        nc.sync.dma_start(out=self.wop[:], in_=dr["wop"][:])
        self.ones = singles.tile([P, 1], BF16, tag="ones", name="ones")
        nc.vector.memset(self.ones[:], 1.0)

        # persistent: kc2[h][:,0,:]=kcr=[Kr;-Kp], [:,1,:]=kcp=[Kp;Kr]
        self.kc2 = [singles.tile([P, 2, S], BF16, tag=f"kc{h}", name=f"kc{h}")
                    for h in range(2)]
        self.qc2 = singles.tile([P, 2, S], BF16, tag="qc2", name="qc2")
        self.vs = [singles.tile([P, SKT, 2 * DK], BF16, tag=f"vs{h}",
                                name=f"vs{h}") for h in range(2)]
        self.xr2hT = singles.tile([P, S], BF16, tag="xr2hT", name="xr2hT")
        self.xp2hT = singles.tile([P, S], BF16, tag="xp2hT", name="xp2hT")

        if "noproj" in self.variant:
            for t in self.kc2 + self.vs + [self.qc2]:
                nc.vector.memset(t[:], 0.01)
        if "noav" in self.variant:
            nc.vector.memset(self.xr2hT[:], 0.01)
            nc.vector.memset(self.xp2hT[:], 0.01)

    def _xdma(self, out, in_):
        if "nodma" not in self.variant:
            self.nc.sync.dma_start(out=out, in_=in_)

    # ---- projections -----------------------------------------------------
    def kq_lead(self):
        """K projection (all strips) + Q projection (strip 0)."""
        if "noproj" in self.variant:
            return
        nc = self.nc
        for s in range(NSTRIP):
            ssl = slice(s * STRIP, (s + 1) * STRIP)
            xtr = self.xpool.tile([P, DT, STRIP], BF16, tag="xs", name="xs")
            self._xdma(xtr[:], self.dr["xkr"].rearrange(
                "(dt p) s -> p dt s", p=P)[:, :, ssl])
            xtp = self.xpool.tile([P, DT, STRIP], BF16, tag="xs", name="xs")
            self._xdma(xtp[:], self.dr["xkp"].rearrange(
                "(dt p) s -> p dt s", p=P)[:, :, ssl])
            for h in range(2):
                hsl = slice(h * DK, (h + 1) * DK)
                pkr = self.psA.tile([P, STRIP], F32, tag="psA", name="psA")
                pkp = self.psA.tile([P, STRIP], F32, tag="psA", name="psA")
                for dt in range(DT):
                    st = (dt == 0)
                    sp = (dt == DT - 1)
                    # kcr = [Kr ; -Kp]
                    nc.tensor.matmul(pkr[0:DK, :], self.wsb["wk"][:, dt, hsl],
                                     xtr[:, dt, :], start=st, stop=sp)
                    nc.tensor.matmul(pkr[DK:P, :], self.wkn[:, dt, hsl],
                                     xtp[:, dt, :], start=st, stop=sp)
                    # kcp = [Kp ; Kr]
                    nc.tensor.matmul(pkp[0:DK, :], self.wsb["wk"][:, dt, hsl],
                                     xtp[:, dt, :], start=st, stop=sp)
                    nc.tensor.matmul(pkp[DK:P, :], self.wsb["wk"][:, dt, hsl],
                                     xtr[:, dt, :], start=st, stop=sp)
                nc.vector.tensor_copy(self.kc2[h][:, 0, ssl], pkr[:])
                nc.vector.tensor_copy(self.kc2[h][:, 1, ssl], pkp[:])
        self.qproj(0)

    def qproj(self, s):
        if "noproj" in self.variant:
            return
        nc = self.nc
        ssl = slice(s * STRIP, (s + 1) * STRIP)
        xtr = self.xpool.tile([P, DT, STRIP], BF16, tag="xs", name="xs")
        self._xdma(xtr[:], self.dr["xqr"].rearrange(
            "(dt p) s -> p dt s", p=P)[:, :, ssl])
        xtp = self.xpool.tile([P, DT, STRIP], BF16, tag="xs", name="xs")
        self._xdma(xtp[:], self.dr["xqp"].rearrange(
            "(dt p) s -> p dt s", p=P)[:, :, ssl])
        for h in range(2):
            hsl = slice(h * DK, (h + 1) * DK)
            pq = self.psA.tile([P, STRIP], F32, tag="psA", name="psA")
            for dt in range(DT):
                st = (dt == 0)
                sp = (dt == DT - 1)
                nc.tensor.matmul(pq[0:DK, :], self.wsb["wq"][:, dt, hsl],
                                 xtr[:, dt, :], start=st, stop=sp)
                nc.tensor.matmul(pq[DK:P, :], self.wsb["wq"][:, dt, hsl],
                                 xtp[:, dt, :], start=st, stop=sp)
            nc.vector.tensor_copy(self.qc2[:, h, ssl], pq[:])

    def vproj(self):
        if "noproj" in self.variant:
            return
        nc = self.nc
        for s in range(NSTRIP):
            ts = slice(s * (STRIP // P), (s + 1) * (STRIP // P))
            for kind, srcn in ((0, "xvr"), (1, "xvp")):
                xt = self.xpool.tile([P, DT, STRIP], BF16, tag="xs", name="xs")
                self._xdma(xt[:], self.dr[srcn].rearrange(
                    "(dt p) s -> p dt s", p=P)[:, :, s * STRIP:(s + 1) * STRIP])
                vv = self.psA.tile([P, STRIP // P, P], F32, tag="psA",
                                   name="psA")
                for tt in range(STRIP // P):
                    for dt in range(DT):
                        nc.tensor.matmul(vv[:, tt, :],
                                         xt[:, dt, tt * P:(tt + 1) * P],
                                         self.wsb["wv"][:, dt, :],
                                         start=(dt == 0), stop=(dt == DT - 1))
                # vs[0] = [vr_h0 | vp_h0] ; vs[1] = [vp_h1 | vr_h1]
                nc.vector.tensor_copy(
                    self.vs[0][:, ts, kind * DK:(kind + 1) * DK],
                    vv[:, :, 0:DK])
                nc.vector.tensor_copy(
                    self.vs[1][:, ts, (1 - kind) * DK:(2 - kind) * DK],
                    vv[:, :, DK:P])

    # ---- attention pipeline ---------------------------------------------
    def _mk_exp(self, u, c):
        def emit():
            csl = slice(c * (STRIP // 4), (c + 1) * (STRIP // 4))
            self.nc.scalar.activation(u[:, :, :, csl], u[:, :, :, csl],
                                      AF.Exp, scale=1.0 / SCALE)
        return emit

    def consume_mms(self, sp, h, j):
        """Interleaved consume slot j (0..SKT-1) for block (sp, h):
        slots 0-7 carry the 16 rowsum matmuls, slots 8-15 the 16 AV."""
        nc, variant = self.nc, self.variant
        st = self.state[sp]
        if j == 0:
            if "nors" not in variant:
                st.setdefault("rs", {})[h] = self.psRS.tile(
                    [1, STRIP], F32, tag="rs", name="rs")
        if j == 0:
            if "noav" not in variant:
                st["av"][h] = self.psAV.tile([P, STRIP], F32, tag="av",
                                             name="av")
        pu = st["u"]
        if j < 8:
            if "nors" in variant:
                return
            for tt in range(2):
                t = j * 2 + tt
                nc.tensor.matmul(st["rs"][h][0:1, :], self.ones[:],
                                 pu[:, h, t, :],
                                 start=(t == 0), stop=(t == SKT - 1),
                                 skip_group_check=True)
        else:
            if j == 8:
                self.recip_bcast(sp, h)
            if "noav" in variant:
                return
            for tt in range(2):
                t = (j - 8) * 2 + tt
                nc.tensor.matmul(st["av"][h][:], self.vs[h][:, t, :],
                                 pu[:, h, t, :],
                                 start=(t == 0), stop=(t == SKT - 1),
                                 skip_group_check=True)
            if j == SKT - 1:
                self.norm(sp, h)

    def recip_bcast(self, sp, h):
        nc = self.nc
        st = self.state[sp]
        rb = self.tpool.tile([P, STRIP], F32, tag="rb", name="rb")
        st["rb"][h] = rb
        if "nors" in self.variant:
            nc.vector.memset(rb[:], 1.0)
        else:
            rrec = self.tpool.tile([1, STRIP], F32, tag="rrec", name="rrec")
            nc.vector.reciprocal_approx_fast(rrec[:], st["rs"][h][0:1, :])
            nc.gpsimd.partition_broadcast(rb[:], rrec[:])

    def norm(self, sp, h):
        nc, variant = self.nc, self.variant
        pssl = slice(sp * STRIP, (sp + 1) * STRIP)
        st = self.state[sp]
        rb = st["rb"][h]
        if "noav" not in variant:
            av = st["av"][h]
            if h == 0:   # av = [xr_h0 ; xp_h0]
                nc.vector.tensor_mul(self.xr2hT[0:DK, pssl], av[0:DK, :],
                                     rb[0:DK, :])
                nc.vector.tensor_mul(self.xp2hT[DK:P, pssl], av[DK:P, :],
                                     rb[DK:P, :])
            else:        # av = [xp_h1 ; xr_h1]
                nc.vector.tensor_mul(self.xp2hT[0:DK, pssl], av[0:DK, :],
                                     rb[0:DK, :])
                nc.vector.tensor_mul(self.xr2hT[DK:P, pssl], av[DK:P, :],
                                     rb[DK:P, :])

    def tail(self, sp):
        """Out-projection for strip sp (both heads already normalised)."""
        nc, variant = self.nc, self.variant
        pssl = slice(sp * STRIP, (sp + 1) * STRIP)
        st = self.state[sp]
        if "noout" in variant:
            return
        for kind, xT, w, out in ((0, self.xr2hT, self.wo, self.dr["o_r"]),
                                 (1, self.xp2hT, self.wop, self.dr["o_p"])):
            for qq in range(STRIP // P):
                q = sp * (STRIP // P) + qq
                qsl = slice(q * P, (q + 1) * P)
                ps_o = self.psAV.tile([P, D], F32, tag="av", name="av")
                nc.tensor.matmul(ps_o[:], xT[:, qsl], w[:], start=True,
                                 stop=True)
                osb = self.opool.tile([P, D], BF16, tag="osb", name="osb")
                if kind == 0:
                    nc.vector.tensor_copy(osb[:], ps_o[:])
                else:
                    nc.scalar.copy(osb[:], ps_o[:])
                nc.sync.dma_start(out=out[qsl, :], in_=osb[:])

    def wrap_prologue(self):
        """Pre-create all per-strip u tiles (static buffer binding across
        For_i iterations) and initialise the two consumed by the first
        iteration's wrapped pipeline stages."""
        self.state = {}
        for s in range(NSTRIP):
            u = self.upool.tile([P, 2, SKT, STRIP], BF16, tag="u", name="u")
            self.state[s] = {"u": u, "av": {}, "rb": {}}
        for s in (NSTRIP - 2, NSTRIP - 1):
            self.nc.vector.memset(self.state[s]["u"][:], 0.25)

    def body(self, trail_proj, wrap):
        nc, variant = self.nc, self.variant
        inv_scale = 1.0 / SCALE
        if not wrap:
            self.state = {}
        self.pending_act = []
        if wrap and "nosqrtexp" not in variant:
            # previous iteration's strip-3 exp chunks run spaced through
            # this iteration's strip 0 (its sqrt ran during the K/Q trail)
            self.pending_act.extend(
                self._mk_exp(self.state[NSTRIP - 1]["u"], c) for c in range(4))

        for s in range(NSTRIP):
            ssl = slice(s * STRIP, (s + 1) * STRIP)
            if wrap:
                sc = (s - 2) % NSTRIP
                u = self.state[s]["u"]
            else:
                sc = s - 2   # strip consumed while strip s computes (2-stage)
                u = self.upool.tile([P, 2, SKT, STRIP], BF16, tag="u",
                                    name="u")
                self.state[s] = {"u": u, "av": {}, "rb": {}}
            if "nosq" in variant:
                nc.vector.memset(u[:], 0.25)
            for h in range(2):
                for t in range(SKT):
                    if h == 0 and t % 4 == 2 and self.pending_act:
                        self.pending_act.pop(0)()
                    if "nosq" not in variant:
                        tsl = slice(t * P, (t + 1) * P)
                        ut = u[:, h, t, :]
                        ps_r = self.psA.tile([P, STRIP], F32, tag="psA",
                                             name="psA")
                        nc.tensor.matmul(ps_r[:], self.kc2[h][:, 0, tsl],
                                         self.qc2[:, h, ssl], start=True,
                                         stop=True)
                        ps_p = self.psA.tile([P, STRIP], F32, tag="psA",
                                             name="psA")
                        nc.tensor.matmul(ps_p[:], self.kc2[h][:, 1, tsl],
                                         self.qc2[:, h, ssl], start=True,
                                         stop=True)
                        if (t * SQT_ACT) % 16 < SQT_ACT:
                            nc.scalar.square(ut, ps_r[:])
                        else:
                            nc.vector._custom_dve(SQ1, out=ut, in0=ps_r[:])
                        nc.vector._custom_dve(SQADD, out=ut, in0=ps_p[:],
                                              in1=ut)
                    if sc is not None and sc >= 0:
                        self.consume_mms(sc, h, t)
            if "nosqrtexp" not in variant:
                # sqrt chunks emitted now (readiness staggers them against
                # the next strip's squares); exp chunks are column-sliced
                # (each depends on ALL sqrt chunks -> no table thrash) and
                # their emission is deferred into the next strip's t-loop
                # so they cannot convoy on the ACT engine.
                for c in range(4):
                    nc.scalar.activation(u[:, :, 4 * c:4 * c + 4, :],
                                         u[:, :, 4 * c:4 * c + 4, :], AF.Sqrt)
                if not (wrap and s == NSTRIP - 1):
                    self.pending_act.extend(
                        self._mk_exp(u, c) for c in range(4))
            if sc is not None and sc >= 0:
                self.tail(sc)
            if s == 0:
                self.vproj()
            if s + 1 < NSTRIP:
                self.qproj(s + 1)

        if wrap:
            # strips 2,3 are consumed by the next iteration's strips 0,1;
            # strip 3's sqrt runs during the K/Q trail, its exp inside the
            # next iteration's strip 0.
            assert not self.pending_act
            if trail_proj:
                self.kq_lead()
            return
        for emit in self.pending_act:
            emit()
        self.pending_act = []
        # next iteration's K/Q(0) projections overlap the drain below
        if trail_proj:
            self.kq_lead()
        for sc in (NSTRIP - 2, NSTRIP - 1):
            for h in range(2):
                for j in range(SKT):
                    self.consume_mms(sc, h, j)
            self.tail(sc)


# ---------------------------------------------------------------------------
_CACHE = {}


def _get_nc(n_iter=1, variant=frozenset()):
    key = (n_iter, variant)
    if key not in _CACHE:
        _CACHE[key] = build(n_iter, variant)
    return _CACHE[key]


def make_in_maps(q_real, k_real, v_real, q_phase, k_phase, v_phase,
                 w_q, w_k, w_v, w_o):
    """Host-side shard + layout prep: per-core input dicts."""
    xt = {}
    for b in range(B):
        xt[("xqr", b)] = np.ascontiguousarray(q_real[b].T).astype(BFNP)
        xt[("xqp", b)] = np.ascontiguousarray(q_phase[b].T).astype(BFNP)
        xt[("xkr", b)] = np.ascontiguousarray(k_real[b].T).astype(BFNP)
        xt[("xkp", b)] = np.ascontiguousarray(k_phase[b].T).astype(BFNP)
        xt[("xvr", b)] = np.ascontiguousarray(v_real[b].T).astype(BFNP)
        xt[("xvp", b)] = np.ascontiguousarray(v_phase[b].T).astype(BFNP)
    wq16, wk16, wv16, wo16 = (w.astype(BFNP) for w in (w_q, w_k, w_v, w_o))
    in_maps = []
    for core in range(N_CORES):
        b, hg = divmod(core, HG)
        csl = slice(hg * 2 * DK, (hg + 1) * 2 * DK)
        wo_c = np.ascontiguousarray(wo16[csl, :])
        wop_c = np.ascontiguousarray(
            np.concatenate([wo_c[DK:2 * DK], wo_c[0:DK]], axis=0))
        in_maps.append({
            "xqr": xt[("xqr", b)], "xqp": xt[("xqp", b)],
            "xkr": xt[("xkr", b)], "xkp": xt[("xkp", b)],
            "xvr": xt[("xvr", b)], "xvp": xt[("xvp", b)],
            "wq": np.ascontiguousarray(wq16[:, csl]),
            "wk": np.ascontiguousarray(wk16[:, csl]),
            "wv": np.ascontiguousarray(wv16[:, csl]),
            "wo": wo_c,
            "wop": wop_c,
        })
    return in_maps


def gather_outputs(results):
    out_r = np.zeros((B, S, D), np.float32)
    out_p = np.zeros((B, S, D), np.float32)
    for core in range(N_CORES):
        b = core // HG
        out_r[b] += np.asarray(results[core]["o_r"], np.float32)
        out_p[b] += np.asarray(results[core]["o_p"], np.float32)
    return out_r, out_p


def _numpy_fallback(q_real, k_real, v_real, q_phase, k_phase, v_phase,
                    w_q, w_k, w_v, w_o, mask):
    def heads(x, w):
        y = x @ w
        return y.reshape(B, -1, H, DK).transpose(0, 2, 1, 3)
    qr, kr, vr = heads(q_real, w_q), heads(k_real, w_k), heads(v_real, w_v)
    qp, kp, vp = heads(q_phase, w_q), heads(k_phase, w_k), heads(v_phase, w_v)
    ar = np.einsum('bhqd,bhkd->bhqk', qr, kr) - np.einsum('bhqd,bhkd->bhqk', qp, kp)
    ap = np.einsum('bhqd,bhkd->bhqk', qr, kp) + np.einsum('bhqd,bhkd->bhqk', qp, kr)
    a = np.sqrt(ar * ar + ap * ap) / SCALE
    a = np.where(mask[:, None, :, :] == 0, np.float32(-1e9), a)
    a = a - a.max(axis=-1, keepdims=True)
    e = np.exp(a)
    a = e / e.sum(axis=-1, keepdims=True)
    xr = np.einsum('bhqk,bhkd->bhqd', a, vr).transpose(0, 2, 1, 3).reshape(B, -1, D)
    xp = np.einsum('bhqk,bhkd->bhqd', a, vp).transpose(0, 2, 1, 3).reshape(B, -1, D)
    return (xr @ w_o).astype(np.float32), (xp @ w_o).astype(np.float32)


def kernel(q_real, k_real, v_real, q_phase, k_phase, v_phase,
           w_q, w_k, w_v, w_o, mask):
    args = [np.asarray(a, np.float32) for a in
            (q_real, k_real, v_real, q_phase, k_phase, v_phase,
             w_q, w_k, w_v, w_o)]
    mask = np.asarray(mask)
    if not np.all(mask != 0):
        return _numpy_fallback(*args, mask)
    nc = _get_nc(1)
    in_maps = make_in_maps(*args)
    res = run_bass_kernel_spmd(nc, in_maps, core_ids=list(range(N_CORES)))
    return gather_outputs(res.results)


# revision 46
# speedup vs baseline: 1.2673x; 1.1918x over previous
"""Trainium2 Bass kernel for nn_MultiHeadAttention_65773129171319.

Complex-valued multi-head attention:
  attn = softmax(|Qc Kc^H| / sqrt(2 dk)) ; out = (attn @ Vr) Wo, (attn @ Vp) Wo

Sharding: 8 cores = 2 (batch) x 4 (head-groups of 2 heads).  Each core
computes its batch's full sequence for its 2 heads; the out-projection
partial sums (over head groups) are reduced on the host.

Device pipeline (per core; strips of 512 q-positions, blocks = (strip, head)):
  - scores come out TRANSPOSED [sk, sq] from stacked-channel matmuls
    (kcr=[Kr;-Kp], kcp=[Kp;Kr] vs qc=[Qr;Qp], contraction 128), in PAIRS of
    sk-tiles sharing a 2-bank PSUM tile to amortise PSUM access latency.
  - u = s_r^2 (ACT Square or DVE SQ1, balanced) then u += s_p^2 (DVE SQADD).
  - sqrt / exp batched per strip on ACT; Square/Sqrt/Copy share one table
    set so only the exp<->sqrt switch reloads tables.
  - consume(strip-1) — 16 rowsum matmuls FIRST (so 1/Z is ready early),
    then 16 merged-AV matmuls (stationary [vr|vp], M=128) — is interleaved
    into the next strip's score matmuls so the PE never idles during the
    ACT/DVE chain; normalisation + out-projection follow.
  - head-1 AV stationary is column-swapped ([vp|vr]) so every DVE op stays
    partition-aligned; the phase out-projection uses a row-swapped Wo.
  - for the repeat-loop build, the K/Q(0) projections are software-pipelined
    ACROSS iterations: emitted once before For_i, then re-emitted at the
    body tail where they overlap the attention drain.
"""

import sys

import numpy as np

try:
    import concourse.bass as bass
except ImportError:  # pragma: no cover
    sys.path.insert(0, "/opt/trn_rl_repo")
    import concourse.bass as bass

import ml_dtypes
import concourse.mybir as mybir
import concourse.tile as tile
from concourse import bacc
from concourse.bass_utils import run_bass_kernel_spmd

B, S, D, H = 2, 2048, 512, 8
DK = D // H  # 64
SCALE = float((2 * DK) ** 0.5)
P = 128
N_CORES = 8
HG = 4            # head groups (2 heads each)
DT = D // P       # 4 d-tiles for projection contraction
SKT = S // P      # 16 sk tiles
NSTRIP = 4        # sq strips of 512
STRIP = S // NSTRIP  # 512
SQT_ACT = 6       # of the 16 sk-tiles per block, how many square on ACT

F32 = mybir.dt.float32
BF16 = mybir.dt.bfloat16
BFNP = ml_dtypes.bfloat16

AF = mybir.ActivationFunctionType


def register_custom_ops():
    """Register fused DVE ops (runtime extension of dve_ops.OPS)."""
    import concourse.dve_ops as dve_ops
    from concourse.dve_ops import DveOp
    from concourse.dve_spec import Spec, Src0, Src1, sq, lower, _has_src1
    from concourse.dve_uop import DveOpSpec

    existing = {op.name: op for op in dve_ops.OPS}

    def mk(name, spec):
        if name in existing:
            return existing[name]
        row = max(dve_ops._SUB_OPCODE_FOR_NAME.values()) + 1
        assert row < 0x20, "no free DVE opcode rows"
        dve_ops._SUB_OPCODE_FOR_NAME[name] = row
        shas = {}
        for ver in ("v3", "v4"):
            s = DveOpSpec(name=name, opcode=row, uops=lower(spec, ver=ver),
                          rd1_en=_has_src1(spec))
            shas[ver] = s.sha(ver)
        op = DveOp(name, spec, subdim=False, uops_sha=shas)
        dve_ops.OPS.append(op)
        return op

    sq1 = mk("SQ1_ANT", Spec(
        body=sq(Src0),
        reference=lambda in0, in1, s0, s1, imm2: in0.astype(np.float32) ** 2))
    sqadd = mk("SQADD_ANT", Spec(
        body=sq(Src0) + Src1,
        reference=lambda in0, in1, s0, s1, imm2:
            in0.astype(np.float32) ** 2 + in1.astype(np.float32)))
    return sq1, sqadd


SQ1, SQADD = register_custom_ops()


def build(n_iter: int = 1, variant: frozenset = frozenset(),
          unroll_wrap: bool = False):
    """Build (and bacc-compile) the per-core SPMD program."""
    nc = bacc.Bacc("TRN2", target_bir_lowering=False, debug=False,
                   num_devices=N_CORES)

    dr = {}
    for name in ("xqr", "xqp", "xkr", "xkp", "xvr", "xvp"):
        dr[name] = nc.dram_tensor(name, [D, S], BF16, kind="ExternalInput")
    for name in ("wq", "wk", "wv"):
        dr[name] = nc.dram_tensor(name, [D, 2 * DK], BF16, kind="ExternalInput")
    dr["wo"] = nc.dram_tensor("wo", [2 * DK, D], BF16, kind="ExternalInput")
    dr["wop"] = nc.dram_tensor("wop", [2 * DK, D], BF16, kind="ExternalInput")
    dr["o_r"] = nc.dram_tensor("o_r", [S, D], BF16, kind="ExternalOutput")
    dr["o_p"] = nc.dram_tensor("o_p", [S, D], BF16, kind="ExternalOutput")

    with tile.TileContext(nc) as tc:
        _emit(tc, dr, n_iter, variant, unroll_wrap)
    nc.compile()
    return nc


def _emit(tc, dr, n_iter, variant=frozenset(), unroll_wrap=False):
    from contextlib import ExitStack

    ctx = ExitStack()
    with ctx:
        pools = dict(
            singles=ctx.enter_context(tc.tile_pool(name="singles", bufs=1)),
            xpool=ctx.enter_context(tc.tile_pool(name="xp", bufs=4)),
            upool=ctx.enter_context(tc.tile_pool(name="up", bufs=4)),
            tpool=ctx.enter_context(tc.tile_pool(name="tp", bufs=2)),
            opool=ctx.enter_context(tc.tile_pool(name="op", bufs=4)),
            psA=ctx.enter_context(tc.tile_pool(name="psA", bufs=4, space="PSUM")),
            psAV=ctx.enter_context(tc.tile_pool(name="psAV", bufs=2, space="PSUM")),
            psRS=ctx.enter_context(tc.tile_pool(name="psRS", bufs=2, space="PSUM")),
        )
        kb = _KernelBody(tc, dr, variant, **pools)
        kb.weights_and_persistent()
        kb.kq_lead()
        if n_iter > 1 and unroll_wrap:
            kb.wrap_prologue()
            for _ in range(n_iter):
                kb.body(trail_proj=True, wrap=True)
        elif n_iter > 1:
            kb.wrap_prologue()
            # unroll inside For_i to amortise its per-iteration all-engine
            # barrier (which would otherwise cut the cross-iteration
            # software pipeline).
            unroll = max(u for u in (4, 3, 2, 1) if n_iter % u == 0)
            with tc.For_i(0, n_iter // unroll, 1):
                for _ in range(unroll):
                    kb.body(trail_proj=True, wrap=True)
        else:
            kb.body(trail_proj=False, wrap=False)


class _KernelBody:
    def __init__(self, tc, dr, variant, singles, xpool, upool, tpool, opool,
                 psA, psAV, psRS):
        self.tc = tc
        self.nc = tc.nc
        self.dr = dr
        self.variant = variant
        self.singles = singles
        self.xpool = xpool
        self.upool = upool
        self.tpool = tpool
        self.opool = opool
        self.psA = psA
        self.psAV = psAV
        self.psRS = psRS

    # ---- one-time setup --------------------------------------------------
    def weights_and_persistent(self):
        nc, dr, singles = self.nc, self.dr, self.singles
        self.wsb = {}
        for name in ("wq", "wk", "wv"):
            t = singles.tile([P, DT, 2 * DK], BF16, tag=f"w_{name}",
                             name=f"w_{name}")
            nc.sync.dma_start(out=t[:],
                              in_=dr[name].rearrange("(dt p) m -> p dt m", p=P))
            self.wsb[name] = t
        self.wkn = singles.tile([P, DT, 2 * DK], BF16, tag="w_wkn", name="w_wkn")
        nc.scalar.mul(out=self.wkn[:], in_=self.wsb["wk"][:], mul=-1.0)
        self.wo = singles.tile([P, D], BF16, tag="w_wo", name="w_wo")
        nc.sync.dma_start(out=self.wo[:], in_=dr["wo"][:])
        self.wop = singles.tile([P, D], BF16, tag="w_wop", name="w_wop")
        nc.sync.dma_start(out=self.wop[:], in_=dr["wop"][:])
        self.ones = singles.tile([P, 1], BF16, tag="ones", name="ones")
        nc.vector.memset(self.ones[:], 1.0)

        # persistent: kc2[h][:,0,:]=kcr=[Kr;-Kp], [:,1,:]=kcp=[Kp;Kr]
        self.kc2 = [singles.tile([P, 2, S], BF16, tag=f"kc{h}", name=f"kc{h}")
                    for h in range(2)]
        self.qc2 = singles.tile([P, 2, S], BF16, tag="qc2", name="qc2")
        self.vs = [singles.tile([P, SKT, 2 * DK], BF16, tag=f"vs{h}",
                                name=f"vs{h}") for h in range(2)]
        self.xr2hT = singles.tile([P, S], BF16, tag="xr2hT", name="xr2hT")
        self.xp2hT = singles.tile([P, S], BF16, tag="xp2hT", name="xp2hT")

        if "noproj" in self.variant:
            for t in self.kc2 + self.vs + [self.qc2]:
                nc.vector.memset(t[:], 0.01)
        if "noav" in self.variant:
            nc.vector.memset(self.xr2hT[:], 0.01)
            nc.vector.memset(self.xp2hT[:], 0.01)

    def _xdma(self, out, in_):
        if "nodma" not in self.variant:
            self.nc.sync.dma_start(out=out, in_=in_)

    # ---- projections -----------------------------------------------------
    def kq_lead(self):
        """K projection (all strips) + Q projection (strip 0)."""
        if "noproj" in self.variant:
            return
        nc = self.nc
        for s in range(NSTRIP):
            ssl = slice(s * STRIP, (s + 1) * STRIP)
            xtr = self.xpool.tile([P, DT, STRIP], BF16, tag="xs", name="xs")
            self._xdma(xtr[:], self.dr["xkr"].rearrange(
                "(dt p) s -> p dt s", p=P)[:, :, ssl])
            xtp = self.xpool.tile([P, DT, STRIP], BF16, tag="xs", name="xs")
            self._xdma(xtp[:], self.dr["xkp"].rearrange(
                "(dt p) s -> p dt s", p=P)[:, :, ssl])
            for h in range(2):
                hsl = slice(h * DK, (h + 1) * DK)
                pkr = self.psA.tile([P, STRIP], F32, tag="psA", name="psA")
                pkp = self.psA.tile([P, STRIP], F32, tag="psA", name="psA")
                for dt in range(DT):
                    st = (dt == 0)
                    sp = (dt == DT - 1)
                    # kcr = [Kr ; -Kp]
                    nc.tensor.matmul(pkr[0:DK, :], self.wsb["wk"][:, dt, hsl],
                                     xtr[:, dt, :], start=st, stop=sp)
                    nc.tensor.matmul(pkr[DK:P, :], self.wkn[:, dt, hsl],
                                     xtp[:, dt, :], start=st, stop=sp)
                    # kcp = [Kp ; Kr]
                    nc.tensor.matmul(pkp[0:DK, :], self.wsb["wk"][:, dt, hsl],
                                     xtp[:, dt, :], start=st, stop=sp)
                    nc.tensor.matmul(pkp[DK:P, :], self.wsb["wk"][:, dt, hsl],
                                     xtr[:, dt, :], start=st, stop=sp)
                nc.vector.tensor_copy(self.kc2[h][:, 0, ssl], pkr[:])
                nc.vector.tensor_copy(self.kc2[h][:, 1, ssl], pkp[:])
        self.qproj(0)

    def qproj(self, s):
        if "noproj" in self.variant:
            return
        nc = self.nc
        ssl = slice(s * STRIP, (s + 1) * STRIP)
        xtr = self.xpool.tile([P, DT, STRIP], BF16, tag="xs", name="xs")
        self._xdma(xtr[:], self.dr["xqr"].rearrange(
            "(dt p) s -> p dt s", p=P)[:, :, ssl])
        xtp = self.xpool.tile([P, DT, STRIP], BF16, tag="xs", name="xs")
        self._xdma(xtp[:], self.dr["xqp"].rearrange(
            "(dt p) s -> p dt s", p=P)[:, :, ssl])
        for h in range(2):
            hsl = slice(h * DK, (h + 1) * DK)
            pq = self.psA.tile([P, STRIP], F32, tag="psA", name="psA")
            for dt in range(DT):
                st = (dt == 0)
                sp = (dt == DT - 1)
                nc.tensor.matmul(pq[0:DK, :], self.wsb["wq"][:, dt, hsl],
                                 xtr[:, dt, :], start=st, stop=sp)
                nc.tensor.matmul(pq[DK:P, :], self.wsb["wq"][:, dt, hsl],
                                 xtp[:, dt, :], start=st, stop=sp)
            nc.vector.tensor_copy(self.qc2[:, h, ssl], pq[:])

    def vproj(self):
        if "noproj" in self.variant:
            return
        nc = self.nc
        for s in range(NSTRIP):
            ts = slice(s * (STRIP // P), (s + 1) * (STRIP // P))
            for kind, srcn in ((0, "xvr"), (1, "xvp")):
                xt = self.xpool.tile([P, DT, STRIP], BF16, tag="xs", name="xs")
                self._xdma(xt[:], self.dr[srcn].rearrange(
                    "(dt p) s -> p dt s", p=P)[:, :, s * STRIP:(s + 1) * STRIP])
                vv = self.psA.tile([P, STRIP // P, P], F32, tag="psA",
                                   name="psA")
                for tt in range(STRIP // P):
                    for dt in range(DT):
                        nc.tensor.matmul(vv[:, tt, :],
                                         xt[:, dt, tt * P:(tt + 1) * P],
                                         self.wsb["wv"][:, dt, :],
                                         start=(dt == 0), stop=(dt == DT - 1))
                # vs[0] = [vr_h0 | vp_h0] ; vs[1] = [vp_h1 | vr_h1]
                nc.vector.tensor_copy(
                    self.vs[0][:, ts, kind * DK:(kind + 1) * DK],
                    vv[:, :, 0:DK])
                nc.vector.tensor_copy(
                    self.vs[1][:, ts, (1 - kind) * DK:(2 - kind) * DK],
                    vv[:, :, DK:P])

    # ---- attention pipeline ---------------------------------------------
    def _mk_exp(self, u, c):
        def emit():
            csl = slice(c * (STRIP // 8), (c + 1) * (STRIP // 8))
            self.nc.scalar.activation(u[:, :, :, csl], u[:, :, :, csl],
                                      AF.Exp, scale=1.0 / SCALE)
        return emit

    def consume_mms(self, sp, h, j):
        """Interleaved consume slot j (0..SKT-1) for block (sp, h):
        slots 0-7 carry the 16 rowsum matmuls, slots 8-15 the 16 AV."""
        nc, variant = self.nc, self.variant
        st = self.state[sp]
        if j == 0:
            if "nors" not in variant:
                st.setdefault("rs", {})[h] = self.psRS.tile(
                    [1, STRIP], F32, tag="rs", name="rs")
        if j == 0:
            if "noav" not in variant:
                st["av"][h] = self.psAV.tile([P, STRIP], F32, tag="av",
                                             name="av")
        pu = st["u"]
        if j < 8:
            if "nors" in variant:
                return
            for tt in range(2):
                t = j * 2 + tt
                nc.tensor.matmul(st["rs"][h][0:1, :], self.ones[:],
                                 pu[:, h, t, :],
                                 start=(t == 0), stop=(t == SKT - 1),
                                 skip_group_check=True)
        else:
            if j == 8:
                self.recip_bcast(sp, h)
            if "noav" in variant:
                return
            for tt in range(2):
                t = (j - 8) * 2 + tt
                nc.tensor.matmul(st["av"][h][:], self.vs[h][:, t, :],
                                 pu[:, h, t, :],
                                 start=(t == 0), stop=(t == SKT - 1),
                                 skip_group_check=True)
            if j == SKT - 1:
                self.norm(sp, h)

    def recip_bcast(self, sp, h):
        nc = self.nc
        st = self.state[sp]
        rb = self.tpool.tile([P, STRIP], F32, tag="rb", name="rb")
        st["rb"][h] = rb
        if "nors" in self.variant:
            nc.vector.memset(rb[:], 1.0)
        else:
            rrec = self.tpool.tile([1, STRIP], F32, tag="rrec", name="rrec")
            nc.vector.reciprocal_approx_fast(rrec[:], st["rs"][h][0:1, :])
            nc.gpsimd.partition_broadcast(rb[:], rrec[:])

    def norm(self, sp, h):
        nc, variant = self.nc, self.variant
        pssl = slice(sp * STRIP, (sp + 1) * STRIP)
        st = self.state[sp]
        rb = st["rb"][h]
        if "noav" not in variant:
            av = st["av"][h]
            if h == 0:   # av = [xr_h0 ; xp_h0]
                nc.vector.tensor_mul(self.xr2hT[0:DK, pssl], av[0:DK, :],
                                     rb[0:DK, :])
                nc.vector.tensor_mul(self.xp2hT[DK:P, pssl], av[DK:P, :],
                                     rb[DK:P, :])
            else:        # av = [xp_h1 ; xr_h1]
                nc.vector.tensor_mul(self.xp2hT[0:DK, pssl], av[0:DK, :],
                                     rb[0:DK, :])
                nc.vector.tensor_mul(self.xr2hT[DK:P, pssl], av[DK:P, :],
                                     rb[DK:P, :])

    def tail(self, sp):
        """Out-projection for strip sp (both heads already normalised)."""
        nc, variant = self.nc, self.variant
        pssl = slice(sp * STRIP, (sp + 1) * STRIP)
        st = self.state[sp]
        if "noout" in variant:
            return
        for kind, xT, w, out in ((0, self.xr2hT, self.wo, self.dr["o_r"]),
                                 (1, self.xp2hT, self.wop, self.dr["o_p"])):
            for qq in range(STRIP // P):
                q = sp * (STRIP // P) + qq
                qsl = slice(q * P, (q + 1) * P)
                ps_o = self.psAV.tile([P, D], F32, tag="av", name="av")
                nc.tensor.matmul(ps_o[:], xT[:, qsl], w[:], start=True,
                                 stop=True)
                osb = self.opool.tile([P, D], BF16, tag="osb", name="osb")
                if kind == 0:
                    nc.vector.tensor_copy(osb[:], ps_o[:])
                else:
                    nc.scalar.copy(osb[:], ps_o[:])
                nc.sync.dma_start(out=out[qsl, :], in_=osb[:])

    def wrap_prologue(self):
        """Pre-create all per-strip u tiles (static buffer binding across
        For_i iterations) and initialise the two consumed by the first
        iteration's wrapped pipeline stages."""
        self.state = {}
        for s in range(NSTRIP):
            u = self.upool.tile([P, 2, SKT, STRIP], BF16, tag="u", name="u")
            self.state[s] = {"u": u, "av": {}, "rb": {}}
        for s in (NSTRIP - 2, NSTRIP - 1):
            self.nc.vector.memset(self.state[s]["u"][:], 0.25)

    def body(self, trail_proj, wrap):
        nc, variant = self.nc, self.variant
        inv_scale = 1.0 / SCALE
        if not wrap:
            self.state = {}
        self.pending_act = []
        if wrap and "nosqrtexp" not in variant:
            # previous iteration's strip-3 exp chunks run spaced through
            # this iteration's strip 0 (its sqrt ran during the K/Q trail)
            self.pending_act.extend(
                self._mk_exp(self.state[NSTRIP - 1]["u"], c)
                for c in range(8))

        for s in range(NSTRIP):
            ssl = slice(s * STRIP, (s + 1) * STRIP)
            if wrap:
                sc = (s - 2) % NSTRIP
                u = self.state[s]["u"]
            else:
                sc = s - 2   # strip consumed while strip s computes (2-stage)
                u = self.upool.tile([P, 2, SKT, STRIP], BF16, tag="u",
                                    name="u")
                self.state[s] = {"u": u, "av": {}, "rb": {}}
            if "nosq" in variant:
                nc.vector.memset(u[:], 0.25)
            for h in range(2):
                for t in range(SKT):
                    if h == 0 and t % (16 // 8) == 1 and self.pending_act:
                        self.pending_act.pop(0)()
                    if "nosq" not in variant:
                        tsl = slice(t * P, (t + 1) * P)
                        ut = u[:, h, t, :]
                        ps_r = self.psA.tile([P, STRIP], F32, tag="psA",
                                             name="psA")
                        nc.tensor.matmul(ps_r[:], self.kc2[h][:, 0, tsl],
                                         self.qc2[:, h, ssl], start=True,
                                         stop=True)
                        ps_p = self.psA.tile([P, STRIP], F32, tag="psA",
                                             name="psA")
                        nc.tensor.matmul(ps_p[:], self.kc2[h][:, 1, tsl],
                                         self.qc2[:, h, ssl], start=True,
                                         stop=True)
                        if (t * SQT_ACT) % 16 < SQT_ACT:
                            nc.scalar.square(ut, ps_r[:])
                        else:
                            nc.vector._custom_dve(SQ1, out=ut, in0=ps_r[:])
                        nc.vector._custom_dve(SQADD, out=ut, in0=ps_p[:],
                                              in1=ut)
                    if sc is not None and sc >= 0:
                        self.consume_mms(sc, h, t)
            if "nosqrtexp" not in variant:
                # sqrt chunks emitted now (readiness staggers them against
                # the next strip's squares); exp chunks are column-sliced
                # (each depends on ALL sqrt chunks -> no table thrash) and
                # their emission is deferred into the next strip's t-loop
                # so they cannot convoy on the ACT engine.
                for c in range(8):
                    w = SKT // 8
                    nc.scalar.activation(u[:, :, w * c:w * c + w, :],
                                         u[:, :, w * c:w * c + w, :], AF.Sqrt)
                if not (wrap and s == NSTRIP - 1):
                    self.pending_act.extend(
                        self._mk_exp(u, c) for c in range(8))
            if sc is not None and sc >= 0:
                self.tail(sc)
            if s == 0:
                self.vproj()
            if s + 1 < NSTRIP:
                self.qproj(s + 1)

        if wrap:
            # strips 2,3 are consumed by the next iteration's strips 0,1;
            # strip 3's sqrt runs during the K/Q trail, its exp inside the
            # next iteration's strip 0.
            assert not self.pending_act
            if trail_proj:
                self.kq_lead()
            return
        for emit in self.pending_act:
            emit()
        self.pending_act = []
        # next iteration's K/Q(0) projections overlap the drain below
        if trail_proj:
            self.kq_lead()
        for sc in (NSTRIP - 2, NSTRIP - 1):
            for h in range(2):
                for j in range(SKT):
                    self.consume_mms(sc, h, j)
            self.tail(sc)


# ---------------------------------------------------------------------------
_CACHE = {}


def _get_nc(n_iter=1, variant=frozenset()):
    key = (n_iter, variant)
    if key not in _CACHE:
        _CACHE[key] = build(n_iter, variant)
    return _CACHE[key]


def make_in_maps(q_real, k_real, v_real, q_phase, k_phase, v_phase,
                 w_q, w_k, w_v, w_o):
    """Host-side shard + layout prep: per-core input dicts."""
    xt = {}
    for b in range(B):
        xt[("xqr", b)] = np.ascontiguousarray(q_real[b].T).astype(BFNP)
        xt[("xqp", b)] = np.ascontiguousarray(q_phase[b].T).astype(BFNP)
        xt[("xkr", b)] = np.ascontiguousarray(k_real[b].T).astype(BFNP)
        xt[("xkp", b)] = np.ascontiguousarray(k_phase[b].T).astype(BFNP)
        xt[("xvr", b)] = np.ascontiguousarray(v_real[b].T).astype(BFNP)
        xt[("xvp", b)] = np.ascontiguousarray(v_phase[b].T).astype(BFNP)
    wq16, wk16, wv16, wo16 = (w.astype(BFNP) for w in (w_q, w_k, w_v, w_o))
    in_maps = []
    for core in range(N_CORES):
        b, hg = divmod(core, HG)
        csl = slice(hg * 2 * DK, (hg + 1) * 2 * DK)
        wo_c = np.ascontiguousarray(wo16[csl, :])
        wop_c = np.ascontiguousarray(
            np.concatenate([wo_c[DK:2 * DK], wo_c[0:DK]], axis=0))
        in_maps.append({
            "xqr": xt[("xqr", b)], "xqp": xt[("xqp", b)],
            "xkr": xt[("xkr", b)], "xkp": xt[("xkp", b)],
            "xvr": xt[("xvr", b)], "xvp": xt[("xvp", b)],
            "wq": np.ascontiguousarray(wq16[:, csl]),
            "wk": np.ascontiguousarray(wk16[:, csl]),
            "wv": np.ascontiguousarray(wv16[:, csl]),
            "wo": wo_c,
            "wop": wop_c,
        })
    return in_maps


def gather_outputs(results):
    out_r = np.zeros((B, S, D), np.float32)
    out_p = np.zeros((B, S, D), np.float32)
    for core in range(N_CORES):
        b = core // HG
        out_r[b] += np.asarray(results[core]["o_r"], np.float32)
        out_p[b] += np.asarray(results[core]["o_p"], np.float32)
    return out_r, out_p


def _numpy_fallback(q_real, k_real, v_real, q_phase, k_phase, v_phase,
                    w_q, w_k, w_v, w_o, mask):
    def heads(x, w):
        y = x @ w
        return y.reshape(B, -1, H, DK).transpose(0, 2, 1, 3)
    qr, kr, vr = heads(q_real, w_q), heads(k_real, w_k), heads(v_real, w_v)
    qp, kp, vp = heads(q_phase, w_q), heads(k_phase, w_k), heads(v_phase, w_v)
    ar = np.einsum('bhqd,bhkd->bhqk', qr, kr) - np.einsum('bhqd,bhkd->bhqk', qp, kp)
    ap = np.einsum('bhqd,bhkd->bhqk', qr, kp) + np.einsum('bhqd,bhkd->bhqk', qp, kr)
    a = np.sqrt(ar * ar + ap * ap) / SCALE
    a = np.where(mask[:, None, :, :] == 0, np.float32(-1e9), a)
    a = a - a.max(axis=-1, keepdims=True)
    e = np.exp(a)
    a = e / e.sum(axis=-1, keepdims=True)
    xr = np.einsum('bhqk,bhkd->bhqd', a, vr).transpose(0, 2, 1, 3).reshape(B, -1, D)
    xp = np.einsum('bhqk,bhkd->bhqd', a, vp).transpose(0, 2, 1, 3).reshape(B, -1, D)
    return (xr @ w_o).astype(np.float32), (xp @ w_o).astype(np.float32)


def kernel(q_real, k_real, v_real, q_phase, k_phase, v_phase,
           w_q, w_k, w_v, w_o, mask):
    args = [np.asarray(a, np.float32) for a in
            (q_real, k_real, v_real, q_phase, k_phase, v_phase,
             w_q, w_k, w_v, w_o)]
    mask = np.asarray(mask)
    if not np.all(mask != 0):
        return _numpy_fallback(*args, mask)
    nc = _get_nc(1)
    in_maps = make_in_maps(*args)
    res = run_bass_kernel_spmd(nc, in_maps, core_ids=list(range(N_CORES)))
    return gather_outputs(res.results)


# revision 47
# speedup vs baseline: 1.3691x; 1.0803x over previous
"""Trainium2 Bass kernel for nn_MultiHeadAttention_65773129171319.

Complex-valued multi-head attention:
  attn = softmax(|Qc Kc^H| / sqrt(2 dk)) ; out = (attn @ Vr) Wo, (attn @ Vp) Wo

Sharding: 8 cores = 2 (batch) x 4 (head-groups of 2 heads).  Each core
computes its batch's full sequence for its 2 heads; the out-projection
partial sums (over head groups) are reduced on the host.

Device pipeline (per core; strips of 512 q-positions, blocks = (strip, head)):
  - scores come out TRANSPOSED [sk, sq] from stacked-channel matmuls
    (kcr=[Kr;-Kp], kcp=[Kp;Kr] vs qc=[Qr;Qp], contraction 128), in PAIRS of
    sk-tiles sharing a 2-bank PSUM tile to amortise PSUM access latency.
  - u = s_r^2 (ACT Square or DVE SQ1, balanced) then u += s_p^2 (DVE SQADD).
  - sqrt / exp batched per strip on ACT; Square/Sqrt/Copy share one table
    set so only the exp<->sqrt switch reloads tables.
  - consume(strip-1) — 16 rowsum matmuls FIRST (so 1/Z is ready early),
    then 16 merged-AV matmuls (stationary [vr|vp], M=128) — is interleaved
    into the next strip's score matmuls so the PE never idles during the
    ACT/DVE chain; normalisation + out-projection follow.
  - head-1 AV stationary is column-swapped ([vp|vr]) so every DVE op stays
    partition-aligned; the phase out-projection uses a row-swapped Wo.
  - for the repeat-loop build, the K/Q(0) projections are software-pipelined
    ACROSS iterations: emitted once before For_i, then re-emitted at the
    body tail where they overlap the attention drain.
"""

import sys

import numpy as np

try:
    import concourse.bass as bass
except ImportError:  # pragma: no cover
    sys.path.insert(0, "/opt/trn_rl_repo")
    import concourse.bass as bass

import ml_dtypes
import concourse.mybir as mybir
import concourse.tile as tile
from concourse import bacc
from concourse.bass_utils import run_bass_kernel_spmd

B, S, D, H = 2, 2048, 512, 8
DK = D // H  # 64
SCALE = float((2 * DK) ** 0.5)
P = 128
N_CORES = 8
HG = 4            # head groups (2 heads each)
DT = D // P       # 4 d-tiles for projection contraction
SKT = S // P      # 16 sk tiles
NSTRIP = 4        # sq strips of 512
STRIP = S // NSTRIP  # 512
SQT_ACT = 6       # of the 16 sk-tiles per block, how many square on ACT

F32 = mybir.dt.float32
BF16 = mybir.dt.bfloat16
BFNP = ml_dtypes.bfloat16

AF = mybir.ActivationFunctionType


def register_custom_ops():
    """Register fused DVE ops (runtime extension of dve_ops.OPS)."""
    import concourse.dve_ops as dve_ops
    from concourse.dve_ops import DveOp
    from concourse.dve_spec import Spec, Src0, Src1, sq, lower, _has_src1
    from concourse.dve_uop import DveOpSpec

    existing = {op.name: op for op in dve_ops.OPS}

    def mk(name, spec):
        if name in existing:
            return existing[name]
        row = max(dve_ops._SUB_OPCODE_FOR_NAME.values()) + 1
        assert row < 0x20, "no free DVE opcode rows"
        dve_ops._SUB_OPCODE_FOR_NAME[name] = row
        shas = {}
        for ver in ("v3", "v4"):
            s = DveOpSpec(name=name, opcode=row, uops=lower(spec, ver=ver),
                          rd1_en=_has_src1(spec))
            shas[ver] = s.sha(ver)
        op = DveOp(name, spec, subdim=False, uops_sha=shas)
        dve_ops.OPS.append(op)
        return op

    sq1 = mk("SQ1_ANT", Spec(
        body=sq(Src0),
        reference=lambda in0, in1, s0, s1, imm2: in0.astype(np.float32) ** 2))
    sqadd = mk("SQADD_ANT", Spec(
        body=sq(Src0) + Src1,
        reference=lambda in0, in1, s0, s1, imm2:
            in0.astype(np.float32) ** 2 + in1.astype(np.float32)))
    return sq1, sqadd


SQ1, SQADD = register_custom_ops()


def build(n_iter: int = 1, variant: frozenset = frozenset(),
          unroll_wrap: bool = False):
    """Build (and bacc-compile) the per-core SPMD program."""
    nc = bacc.Bacc("TRN2", target_bir_lowering=False, debug=False,
                   num_devices=N_CORES)

    dr = {}
    for name in ("xqr", "xqp", "xkr", "xkp", "xvr", "xvp"):
        dr[name] = nc.dram_tensor(name, [D, S], BF16, kind="ExternalInput")
    for name in ("wq", "wk", "wv"):
        dr[name] = nc.dram_tensor(name, [D, 2 * DK], BF16, kind="ExternalInput")
    dr["wo"] = nc.dram_tensor("wo", [2 * DK, D], BF16, kind="ExternalInput")
    dr["wop"] = nc.dram_tensor("wop", [2 * DK, D], BF16, kind="ExternalInput")
    dr["o_r"] = nc.dram_tensor("o_r", [S, D], BF16, kind="ExternalOutput")
    dr["o_p"] = nc.dram_tensor("o_p", [S, D], BF16, kind="ExternalOutput")

    with tile.TileContext(nc) as tc:
        _emit(tc, dr, n_iter, variant, unroll_wrap)
    nc.compile()
    return nc


def _emit(tc, dr, n_iter, variant=frozenset(), unroll_wrap=False):
    from contextlib import ExitStack

    ctx = ExitStack()
    with ctx:
        pools = dict(
            singles=ctx.enter_context(tc.tile_pool(name="singles", bufs=1)),
            xpool=ctx.enter_context(tc.tile_pool(name="xp", bufs=4)),
            upool=ctx.enter_context(tc.tile_pool(name="up", bufs=4)),
            tpool=ctx.enter_context(tc.tile_pool(name="tp", bufs=2)),
            opool=ctx.enter_context(tc.tile_pool(name="op", bufs=4)),
            psA=ctx.enter_context(tc.tile_pool(name="psA", bufs=4, space="PSUM")),
            psAV=ctx.enter_context(tc.tile_pool(name="psAV", bufs=2, space="PSUM")),
            psRS=ctx.enter_context(tc.tile_pool(name="psRS", bufs=2, space="PSUM")),
        )
        kb = _KernelBody(tc, dr, variant, **pools)
        kb.weights_and_persistent()
        kb.kq_lead()
        if n_iter > 1 and unroll_wrap:
            kb.wrap_prologue()
            for _ in range(n_iter):
                kb.body(trail_proj=True, wrap=True)
        elif n_iter > 1:
            kb.wrap_prologue()
            # unroll inside For_i to amortise its per-iteration all-engine
            # barrier (which would otherwise cut the cross-iteration
            # software pipeline).
            unroll = max(u for u in (4, 3, 2, 1) if n_iter % u == 0)
            with tc.For_i(0, n_iter // unroll, 1):
                for _ in range(unroll):
                    kb.body(trail_proj=True, wrap=True)
        else:
            kb.body(trail_proj=False, wrap=False)


class _KernelBody:
    def __init__(self, tc, dr, variant, singles, xpool, upool, tpool, opool,
                 psA, psAV, psRS):
        self.tc = tc
        self.nc = tc.nc
        self.dr = dr
        self.variant = variant
        self.singles = singles
        self.xpool = xpool
        self.upool = upool
        self.tpool = tpool
        self.opool = opool
        self.psA = psA
        self.psAV = psAV
        self.psRS = psRS

    # ---- one-time setup --------------------------------------------------
    def weights_and_persistent(self):
        nc, dr, singles = self.nc, self.dr, self.singles
        self.wsb = {}
        for name in ("wq", "wk", "wv"):
            t = singles.tile([P, DT, 2 * DK], BF16, tag=f"w_{name}",
                             name=f"w_{name}")
            nc.sync.dma_start(out=t[:],
                              in_=dr[name].rearrange("(dt p) m -> p dt m", p=P))
            self.wsb[name] = t
        self.wkn = singles.tile([P, DT, 2 * DK], BF16, tag="w_wkn", name="w_wkn")
        nc.scalar.mul(out=self.wkn[:], in_=self.wsb["wk"][:], mul=-1.0)
        self.wo = singles.tile([P, D], BF16, tag="w_wo", name="w_wo")
        nc.sync.dma_start(out=self.wo[:], in_=dr["wo"][:])
        self.wop = singles.tile([P, D], BF16, tag="w_wop", name="w_wop")
        nc.sync.dma_start(out=self.wop[:], in_=dr["wop"][:])
        self.ones = singles.tile([P, 1], BF16, tag="ones", name="ones")
        nc.vector.memset(self.ones[:], 1.0)

        # persistent: kc2[h][:,0,:]=kcr=[Kr;-Kp], [:,1,:]=kcp=[Kp;Kr]
        self.kc2 = [singles.tile([P, 2, S], BF16, tag=f"kc{h}", name=f"kc{h}")
                    for h in range(2)]
        self.qc2 = singles.tile([P, 2, S], BF16, tag="qc2", name="qc2")
        self.vs = [singles.tile([P, SKT, 2 * DK], BF16, tag=f"vs{h}",
                                name=f"vs{h}") for h in range(2)]
        self.xr2hT = singles.tile([P, S], BF16, tag="xr2hT", name="xr2hT")
        self.xp2hT = singles.tile([P, S], BF16, tag="xp2hT", name="xp2hT")

        if "noproj" in self.variant:
            for t in self.kc2 + self.vs + [self.qc2]:
                nc.vector.memset(t[:], 0.01)
        if "noav" in self.variant:
            nc.vector.memset(self.xr2hT[:], 0.01)
            nc.vector.memset(self.xp2hT[:], 0.01)

    def _xdma(self, out, in_):
        if "nodma" not in self.variant:
            self.nc.sync.dma_start(out=out, in_=in_)

    # ---- projections -----------------------------------------------------
    def kq_lead(self):
        """K projection (all strips) + Q projection (strip 0)."""
        if "noproj" in self.variant:
            return
        nc = self.nc
        for s in range(NSTRIP):
            ssl = slice(s * STRIP, (s + 1) * STRIP)
            xtr = self.xpool.tile([P, DT, STRIP], BF16, tag="xs", name="xs")
            self._xdma(xtr[:], self.dr["xkr"].rearrange(
                "(dt p) s -> p dt s", p=P)[:, :, ssl])
            xtp = self.xpool.tile([P, DT, STRIP], BF16, tag="xs", name="xs")
            self._xdma(xtp[:], self.dr["xkp"].rearrange(
                "(dt p) s -> p dt s", p=P)[:, :, ssl])
            for h in range(2):
                hsl = slice(h * DK, (h + 1) * DK)
                pkr = self.psA.tile([P, STRIP], F32, tag="psA", name="psA")
                pkp = self.psA.tile([P, STRIP], F32, tag="psA", name="psA")
                for dt in range(DT):
                    st = (dt == 0)
                    sp = (dt == DT - 1)
                    # kcr = [Kr ; -Kp]
                    nc.tensor.matmul(pkr[0:DK, :], self.wsb["wk"][:, dt, hsl],
                                     xtr[:, dt, :], start=st, stop=sp)
                    nc.tensor.matmul(pkr[DK:P, :], self.wkn[:, dt, hsl],
                                     xtp[:, dt, :], start=st, stop=sp)
                    # kcp = [Kp ; Kr]
                    nc.tensor.matmul(pkp[0:DK, :], self.wsb["wk"][:, dt, hsl],
                                     xtp[:, dt, :], start=st, stop=sp)
                    nc.tensor.matmul(pkp[DK:P, :], self.wsb["wk"][:, dt, hsl],
                                     xtr[:, dt, :], start=st, stop=sp)
                nc.vector.tensor_copy(self.kc2[h][:, 0, ssl], pkr[:])
                nc.vector.tensor_copy(self.kc2[h][:, 1, ssl], pkp[:])
        self.qproj(0)

    def qproj(self, s):
        if "noproj" in self.variant:
            return
        nc = self.nc
        ssl = slice(s * STRIP, (s + 1) * STRIP)
        xtr = self.xpool.tile([P, DT, STRIP], BF16, tag="xs", name="xs")
        self._xdma(xtr[:], self.dr["xqr"].rearrange(
            "(dt p) s -> p dt s", p=P)[:, :, ssl])
        xtp = self.xpool.tile([P, DT, STRIP], BF16, tag="xs", name="xs")
        self._xdma(xtp[:], self.dr["xqp"].rearrange(
            "(dt p) s -> p dt s", p=P)[:, :, ssl])
        for h in range(2):
            hsl = slice(h * DK, (h + 1) * DK)
            pq = self.psA.tile([P, STRIP], F32, tag="psA", name="psA")
            for dt in range(DT):
                st = (dt == 0)
                sp = (dt == DT - 1)
                nc.tensor.matmul(pq[0:DK, :], self.wsb["wq"][:, dt, hsl],
                                 xtr[:, dt, :], start=st, stop=sp)
                nc.tensor.matmul(pq[DK:P, :], self.wsb["wq"][:, dt, hsl],
                                 xtp[:, dt, :], start=st, stop=sp)
            nc.vector.tensor_copy(self.qc2[:, h, ssl], pq[:])

    def vproj(self):
        if "noproj" in self.variant:
            return
        nc = self.nc
        for s in range(NSTRIP):
            ts = slice(s * (STRIP // P), (s + 1) * (STRIP // P))
            for kind, srcn in ((0, "xvr"), (1, "xvp")):
                xt = self.xpool.tile([P, DT, STRIP], BF16, tag="xs", name="xs")
                self._xdma(xt[:], self.dr[srcn].rearrange(
                    "(dt p) s -> p dt s", p=P)[:, :, s * STRIP:(s + 1) * STRIP])
                vv = self.psA.tile([P, STRIP // P, P], F32, tag="psA",
                                   name="psA")
                for tt in range(STRIP // P):
                    for dt in range(DT):
                        nc.tensor.matmul(vv[:, tt, :],
                                         xt[:, dt, tt * P:(tt + 1) * P],
                                         self.wsb["wv"][:, dt, :],
                                         start=(dt == 0), stop=(dt == DT - 1))
                # vs[0] = [vr_h0 | vp_h0] ; vs[1] = [vp_h1 | vr_h1]
                nc.vector.tensor_copy(
                    self.vs[0][:, ts, kind * DK:(kind + 1) * DK],
                    vv[:, :, 0:DK])
                nc.vector.tensor_copy(
                    self.vs[1][:, ts, (1 - kind) * DK:(2 - kind) * DK],
                    vv[:, :, DK:P])

    # ---- attention pipeline ---------------------------------------------
    def _mk_exp(self, u, c):
        def emit():
            csl = slice(c * (STRIP // 8), (c + 1) * (STRIP // 8))
            self.nc.scalar.activation(u[:, :, :, csl], u[:, :, :, csl],
                                      AF.Exp, scale=1.0 / SCALE)
        return emit

    def consume_mms(self, sp, h, j):
        """Interleaved consume slot j (0..SKT-1) for block (sp, h):
        slots 0-7 carry the 16 rowsum matmuls, slots 8-15 the 16 AV."""
        nc, variant = self.nc, self.variant
        st = self.state[sp]
        if j == 0:
            if "nors" not in variant:
                st.setdefault("rs", {})[h] = self.psRS.tile(
                    [1, STRIP], F32, tag="rs", name="rs")
        if j == 0:
            if "noav" not in variant:
                st["av"][h] = self.psAV.tile([P, STRIP], F32, tag="av",
                                             name="av")
        pu = st["u"]
        if j < 8:
            if "nors" in variant:
                return
            for tt in range(2):
                t = j * 2 + tt
                nc.tensor.matmul(st["rs"][h][0:1, :], self.ones[:],
                                 pu[:, h, t, :],
                                 start=(t == 0), stop=(t == SKT - 1),
                                 skip_group_check=True)
        else:
            if j == 8:
                self.recip_bcast(sp, h)
            if "noav" in variant:
                return
            for tt in range(2):
                t = (j - 8) * 2 + tt
                nc.tensor.matmul(st["av"][h][:], self.vs[h][:, t, :],
                                 pu[:, h, t, :],
                                 start=(t == 0), stop=(t == SKT - 1),
                                 skip_group_check=True)
            if j == SKT - 1:
                self.norm(sp, h)

    def recip_bcast(self, sp, h):
        nc = self.nc
        st = self.state[sp]
        rb = self.tpool.tile([P, STRIP], F32, tag="rb", name="rb")
        st["rb"][h] = rb
        if "nors" in self.variant:
            nc.vector.memset(rb[:], 1.0)
        else:
            rrec = self.tpool.tile([1, STRIP], F32, tag="rrec", name="rrec")
            nc.vector.reciprocal_approx_fast(rrec[:], st["rs"][h][0:1, :])
            nc.gpsimd.partition_broadcast(rb[:], rrec[:])

    def norm(self, sp, h):
        nc, variant = self.nc, self.variant
        pssl = slice(sp * STRIP, (sp + 1) * STRIP)
        st = self.state[sp]
        rb = st["rb"][h]
        if "noav" not in variant:
            av = st["av"][h]
            if h == 0:   # av = [xr_h0 ; xp_h0]
                nc.vector.tensor_mul(self.xr2hT[0:DK, pssl], av[0:DK, :],
                                     rb[0:DK, :])
                nc.vector.tensor_mul(self.xp2hT[DK:P, pssl], av[DK:P, :],
                                     rb[DK:P, :])
            else:        # av = [xp_h1 ; xr_h1]
                nc.vector.tensor_mul(self.xp2hT[0:DK, pssl], av[0:DK, :],
                                     rb[0:DK, :])
                nc.vector.tensor_mul(self.xr2hT[DK:P, pssl], av[DK:P, :],
                                     rb[DK:P, :])

    def tail(self, sp):
        """Out-projection for strip sp (both heads already normalised)."""
        nc, variant = self.nc, self.variant
        pssl = slice(sp * STRIP, (sp + 1) * STRIP)
        st = self.state[sp]
        if "noout" in variant:
            return
        for kind, xT, w, out in ((0, self.xr2hT, self.wo, self.dr["o_r"]),
                                 (1, self.xp2hT, self.wop, self.dr["o_p"])):
            for qq in range(STRIP // P):
                q = sp * (STRIP // P) + qq
                qsl = slice(q * P, (q + 1) * P)
                ps_o = self.psAV.tile([P, D], F32, tag="av", name="av")
                nc.tensor.matmul(ps_o[:], xT[:, qsl], w[:], start=True,
                                 stop=True)
                osb = self.opool.tile([P, D], BF16, tag="osb", name="osb")
                if kind == 0:
                    nc.vector.tensor_copy(osb[:], ps_o[:])
                else:
                    nc.scalar.copy(osb[:], ps_o[:])
                nc.sync.dma_start(out=out[qsl, :], in_=osb[:])

    def wrap_prologue(self):
        """Pre-create all per-strip u tiles (static buffer binding across
        For_i iterations) and initialise the two consumed by the first
        iteration's wrapped pipeline stages."""
        self.state = {}
        for s in range(NSTRIP):
            u = self.upool.tile([P, 2, SKT, STRIP], BF16, tag="u", name="u")
            self.state[s] = {"u": u, "av": {}, "rb": {}}
        for s in (NSTRIP - 2, NSTRIP - 1):
            self.nc.vector.memset(self.state[s]["u"][:], 0.25)

    def body(self, trail_proj, wrap):
        nc, variant = self.nc, self.variant
        inv_scale = 1.0 / SCALE
        if not wrap:
            self.state = {}
        self.pending_act = []
        if wrap and "nosqrtexp" not in variant:
            # previous iteration's strip-3 exp chunks run spaced through
            # this iteration's strip 0 (its sqrt ran during the K/Q trail)
            self.pending_act.extend(
                self._mk_exp(self.state[NSTRIP - 1]["u"], c)
                for c in range(8))

        for s in range(NSTRIP):
            ssl = slice(s * STRIP, (s + 1) * STRIP)
            if wrap:
                sc = (s - 2) % NSTRIP
                u = self.state[s]["u"]
            else:
                sc = s - 2   # strip consumed while strip s computes (2-stage)
                u = self.upool.tile([P, 2, SKT, STRIP], BF16, tag="u",
                                    name="u")
                self.state[s] = {"u": u, "av": {}, "rb": {}}
            if "nosq" in variant:
                nc.vector.memset(u[:], 0.25)
            for h in range(2):
                for t in range(SKT):
                    if t % 4 == 1 and self.pending_act:
                        self.pending_act.pop(0)()
                    if "nosq" not in variant:
                        tsl = slice(t * P, (t + 1) * P)
                        ut = u[:, h, t, :]
                        ps_r = self.psA.tile([P, STRIP], F32, tag="psA",
                                             name="psA")
                        nc.tensor.matmul(ps_r[:], self.kc2[h][:, 0, tsl],
                                         self.qc2[:, h, ssl], start=True,
                                         stop=True)
                        ps_p = self.psA.tile([P, STRIP], F32, tag="psA",
                                             name="psA")
                        nc.tensor.matmul(ps_p[:], self.kc2[h][:, 1, tsl],
                                         self.qc2[:, h, ssl], start=True,
                                         stop=True)
                        if (t * SQT_ACT) % 16 < SQT_ACT:
                            nc.scalar.square(ut, ps_r[:])
                        else:
                            nc.vector._custom_dve(SQ1, out=ut, in0=ps_r[:])
                        nc.vector._custom_dve(SQADD, out=ut, in0=ps_p[:],
                                              in1=ut)
                    if sc is not None and sc >= 0:
                        self.consume_mms(sc, h, t)
            if "nosqrtexp" not in variant:
                # sqrt chunks emitted now (readiness staggers them against
                # the next strip's squares); exp chunks are column-sliced
                # (each depends on ALL sqrt chunks -> no table thrash) and
                # their emission is deferred into the next strip's t-loop
                # so they cannot convoy on the ACT engine.
                for c in range(8):
                    w = SKT // 8
                    nc.scalar.activation(u[:, :, w * c:w * c + w, :],
                                         u[:, :, w * c:w * c + w, :], AF.Sqrt)
                if not (wrap and s == NSTRIP - 1):
                    self.pending_act.extend(
                        self._mk_exp(u, c) for c in range(8))
            if sc is not None and sc >= 0:
                self.tail(sc)
            if s == 0:
                self.vproj()
            if s + 1 < NSTRIP:
                self.qproj(s + 1)

        if wrap:
            # strips 2,3 are consumed by the next iteration's strips 0,1;
            # strip 3's sqrt runs during the K/Q trail, its exp inside the
            # next iteration's strip 0.
            assert not self.pending_act
            if trail_proj:
                self.kq_lead()
            return
        for emit in self.pending_act:
            emit()
        self.pending_act = []
        # next iteration's K/Q(0) projections overlap the drain below
        if trail_proj:
            self.kq_lead()
        for sc in (NSTRIP - 2, NSTRIP - 1):
            for h in range(2):
                for j in range(SKT):
                    self.consume_mms(sc, h, j)
            self.tail(sc)


# ---------------------------------------------------------------------------
_CACHE = {}


def _get_nc(n_iter=1, variant=frozenset()):
    key = (n_iter, variant)
    if key not in _CACHE:
        _CACHE[key] = build(n_iter, variant)
    return _CACHE[key]


def make_in_maps(q_real, k_real, v_real, q_phase, k_phase, v_phase,
                 w_q, w_k, w_v, w_o):
    """Host-side shard + layout prep: per-core input dicts."""
    xt = {}
    for b in range(B):
        xt[("xqr", b)] = np.ascontiguousarray(q_real[b].T).astype(BFNP)
        xt[("xqp", b)] = np.ascontiguousarray(q_phase[b].T).astype(BFNP)
        xt[("xkr", b)] = np.ascontiguousarray(k_real[b].T).astype(BFNP)
        xt[("xkp", b)] = np.ascontiguousarray(k_phase[b].T).astype(BFNP)
        xt[("xvr", b)] = np.ascontiguousarray(v_real[b].T).astype(BFNP)
        xt[("xvp", b)] = np.ascontiguousarray(v_phase[b].T).astype(BFNP)
    wq16, wk16, wv16, wo16 = (w.astype(BFNP) for w in (w_q, w_k, w_v, w_o))
    in_maps = []
    for core in range(N_CORES):
        b, hg = divmod(core, HG)
        csl = slice(hg * 2 * DK, (hg + 1) * 2 * DK)
        wo_c = np.ascontiguousarray(wo16[csl, :])
        wop_c = np.ascontiguousarray(
            np.concatenate([wo_c[DK:2 * DK], wo_c[0:DK]], axis=0))
        in_maps.append({
            "xqr": xt[("xqr", b)], "xqp": xt[("xqp", b)],
            "xkr": xt[("xkr", b)], "xkp": xt[("xkp", b)],
            "xvr": xt[("xvr", b)], "xvp": xt[("xvp", b)],
            "wq": np.ascontiguousarray(wq16[:, csl]),
            "wk": np.ascontiguousarray(wk16[:, csl]),
            "wv": np.ascontiguousarray(wv16[:, csl]),
            "wo": wo_c,
            "wop": wop_c,
        })
    return in_maps


def gather_outputs(results):
    out_r = np.zeros((B, S, D), np.float32)
    out_p = np.zeros((B, S, D), np.float32)
    for core in range(N_CORES):
        b = core // HG
        out_r[b] += np.asarray(results[core]["o_r"], np.float32)
        out_p[b] += np.asarray(results[core]["o_p"], np.float32)
    return out_r, out_p


def _numpy_fallback(q_real, k_real, v_real, q_phase, k_phase, v_phase,
                    w_q, w_k, w_v, w_o, mask):
    def heads(x, w):
        y = x @ w
        return y.reshape(B, -1, H, DK).transpose(0, 2, 1, 3)
    qr, kr, vr = heads(q_real, w_q), heads(k_real, w_k), heads(v_real, w_v)
    qp, kp, vp = heads(q_phase, w_q), heads(k_phase, w_k), heads(v_phase, w_v)
    ar = np.einsum('bhqd,bhkd->bhqk', qr, kr) - np.einsum('bhqd,bhkd->bhqk', qp, kp)
    ap = np.einsum('bhqd,bhkd->bhqk', qr, kp) + np.einsum('bhqd,bhkd->bhqk', qp, kr)
    a = np.sqrt(ar * ar + ap * ap) / SCALE
    a = np.where(mask[:, None, :, :] == 0, np.float32(-1e9), a)
    a = a - a.max(axis=-1, keepdims=True)
    e = np.exp(a)
    a = e / e.sum(axis=-1, keepdims=True)
    xr = np.einsum('bhqk,bhkd->bhqd', a, vr).transpose(0, 2, 1, 3).reshape(B, -1, D)
    xp = np.einsum('bhqk,bhkd->bhqd', a, vp).transpose(0, 2, 1, 3).reshape(B, -1, D)
    return (xr @ w_o).astype(np.float32), (xp @ w_o).astype(np.float32)


def kernel(q_real, k_real, v_real, q_phase, k_phase, v_phase,
           w_q, w_k, w_v, w_o, mask):
    args = [np.asarray(a, np.float32) for a in
            (q_real, k_real, v_real, q_phase, k_phase, v_phase,
             w_q, w_k, w_v, w_o)]
    mask = np.asarray(mask)
    if not np.all(mask != 0):
        return _numpy_fallback(*args, mask)
    nc = _get_nc(1)
    in_maps = make_in_maps(*args)
    res = run_bass_kernel_spmd(nc, in_maps, core_ids=list(range(N_CORES)))
    return gather_outputs(res.results)
